# revision 25
# baseline (speedup 1.0000x reference)
"""Trainium2 Bass kernel for nn_Block_72138270704025 (windowed sparse attention
block: LN1 -> window partition -> MHA with decomposed rel-pos bias gathered by
q_idx/k_idx -> window unpartition -> residual -> LN2 -> MLP(gelu) -> residual).

Sharding: data-parallel over batch B=8, one batch element per NeuronCore; all
weights replicated.  Host folds LN affine params into the adjacent matmul
weights, precomputes the rel-pos tables Sh/Sw, and turns the per-(window,head)
index gathers into small per-window fp8 operands so the bias folds into the
logits matmul.

v3 changes over v2:
- Merged QK+bias logits: per head the contraction is the 96-row concat
  [k_h(64); Ek_h(28+4 zero pad)] x [q_h(64); Eq_h(32)], so ONE matmul per
  (head, key-chunk) produces logits+bias (v2 used two).  Halves the logits
  matmul and ldweights count.  Odd heads' q/k are evacuated from PSUM rows
  64:128 to SBUF rows 0:64 with a partition-shifted DVE op (64-channel DVE
  ops may read any source partition window); even heads evacuate on ACT.
- Phase C (residual+LN2+MLP) groups deferred one band and interleaved
  BETWEEN window groups, so band boundaries always have ready PE work and
  the serial scatter->load->LN2 chain of a fresh c-group is hidden.
- xn1 and attn intermediates stored as zero-padded 70x70 images in DRAM:
  every window gather/scatter is 2 composite 3D DMAs (v2 used per-row DMAs
  plus memsets for the 9 edge windows), shortening the GpSimd DMA queue.

Carried over from v2: fp8e4m3 DoubleRow matmuls for qkv/v/proj/fc1 and PV
(weights x32 host scale), softmax reciprocal per head-pair, head software
pipelining, transposes as identity matmuls, batched LN sqrt, bf16 fc2.
"""
import os
import sys

for _p in ('/opt/trn_rl_repo', '/root/.axon_site/_ro/trn_rl_repo'):
    if os.path.isdir(_p) and _p not in sys.path:
        sys.path.append(_p)

import numpy as np
import ml_dtypes

import concourse.bass as bass
import concourse.tile as tile
from concourse import mybir
from concourse.bass_utils import run_bass_kernel_spmd
from concourse.tile import ScopedClock
from concourse.masks import make_identity

# ---- problem constants (hardcoded per contest rules) ----
B = 8
HH = 64
WW = 64
DIM = 768
NH = 12
WS = 14
HD = 64
N = 196            # tokens per window
NWS = 5            # window grid side
NW = 25            # windows per image
EPS = 1e-5
NTOK = HH * WW     # 4096 tokens per core
CH = 98            # window token chunk: 7 rows of 14 (196 = 2x98)
WSCALE = 32.0      # host fp8 weight scale (descaled at PSUM evacuation)
HP = 70            # padded image side (5 windows x 14)
EB = 32            # bias contraction rows (28 used + 4 zero pad)

F32 = mybir.dt.float32
BF16 = mybir.dt.bfloat16
F8 = mybir.dt.float8e4
DR = mybir.MatmulPerfMode.DoubleRow


def _patch_tile_drain():
    """Walrus CoreV3 codegen rejects a Drain carrying multiple sem waits
    ("Too many sync wait commands").  Emit explicit wait_ge instructions
    before the kernel-tail drain instead."""
    if getattr(tile.TileContext, '_drain_patched', False):
        return

    def _drain_and_barrier(self, tick_clock, wait_clock):
        nc = self.nc
        dummy = nc.sync.nop(nofuse=True)
        wait_clock.add_sem_waits(dummy.ins, ScopedClock({None: tick_clock.global_clock}))
        waits = list(dummy.ins.sync_info.on_wait or [])
        dummy.ins.sync_info.on_wait = []
        assert self.sems is not None
        by_id = {}
        for h in self.sems.allocated().values():
            by_id[getattr(h, 'id', None)] = h
            by_id[getattr(h, 'name', None)] = h
        for w in waits:
            h = by_id.get(w.id) or by_id.get(w.ant_name)
            assert h is not None, (w.id, w.ant_name)
            nc.sync.wait_ge(h, w.wait_value)
        nc.sync.drain()
        nc.all_engine_barrier()
        popped = nc._tile_sem_poison_stack.pop()
        assert popped is self._sem_poison
        nc.clear_and_free_semaphores(list(self.sems.allocated().values()))
        nc.all_engine_barrier()

    tile.TileContext._drain_and_barrier = _drain_and_barrier
    tile.TileContext._drain_patched = True


def _install_ntff_hook():
    """Recreate the missing antenv.axon_hooks module so trace=True can reach
    the axon NTFF profiler (used only when KERNEL_TRACE=1)."""
    try:
        import types
        import antenv
        if 'antenv.axon_hooks' in sys.modules:
            return True
        mod = types.ModuleType('antenv.axon_hooks')
        mod._hook = None
        mod.set_axon_ntff_profile_hook = lambda h: setattr(mod, '_hook', h)
        mod.get_axon_ntff_profile_hook = lambda: mod._hook
        sys.modules['antenv.axon_hooks'] = mod
        antenv.axon_hooks = mod
        from trn_agent_boot.trn_boot import _ntff_profile_via_ctypes
        mod._hook = _ntff_profile_via_ctypes('/opt/axon/libaxon_pjrt.so')
        return mod._hook is not None
    except Exception:
        return False


# window geometry helpers
def _win_rc(w):
    return w // NWS, w % NWS


_CACHE = {}


def _dedup_ldweights(nc):
    """Tile lowers each matmul to Ldweights+Matmult.  Back-to-back matmuls
    that share a stationary operand reload identical weights; drop the
    redundant Ldweights (keeping its sem waits / updates on a zero-cost
    EventSemaphore)."""
    ndrop = 0
    for fn in nc.m.functions:
        for blk in fn.blocks:
            insts = blk.instructions
            out = []
            prev_key = None
            dirty = False
            for ins in insts:
                if ins.engine != mybir.EngineType.PE:
                    out.append(ins)
                    continue
                if ins.opcode == 'Ldweights':
                    a = ins.ins[0]
                    key = (str(getattr(a, 'memory_location', None)),
                           getattr(a, 'offset', None), str(getattr(a, 'ap', None)),
                           str(getattr(ins, 'is_transpose', None)),
                           str(getattr(ins, 'perf_mode', None)))
                    si = ins.sync_info
                    has_sync = si and (si.on_wait or si.on_update)
                    if key == prev_key:
                        ndrop += 1
                        dirty = True
                        if has_sync:
                            ev = mybir.InstEventSemaphore(
                                name=f"LDDROP-{nc.next_id()}", ins=[], outs=[])
                            ev.engine = ins.engine
                            ev.sync_info = mybir.SyncInfo(
                                on_wait=list(si.on_wait or []),
                                on_update=list(si.on_update or []))
                            out.append(ev)
                        continue
                    prev_key = key
                    out.append(ins)
                elif ins.opcode == 'Matmult' and not getattr(ins, 'is_transpose', False):
                    out.append(ins)
                else:
                    prev_key = None
                    out.append(ins)
            if dirty:
                blk.instructions = out
    return ndrop


def _split_waits(nc, cap=None):
    """Walrus CoreV2/V3 codegen rejects instructions whose sync_info carries
    more waits than the per-opcode ISA ctrl struct holds.  Hoist excess waits
    onto standalone EventSemaphore instructions."""
    if cap is None:
        cap = int(os.environ.get('KERNEL_MAXWAITS', '1'))
    n_split = 0
    for fn in nc.m.functions:
        for blk in fn.blocks:
            insts = blk.instructions
            out = []
            dirty = False
            for ins in insts:
                si = ins.sync_info
                waits = list(si.on_wait) if si and si.on_wait else []
                # InstISA (custom DVE ops) cannot encode sem waits at all
                limit = 0 if ins.opcode == 'ISA' else (
                    1 if ins.opcode in ('Drain',) else cap)
                if len(waits) > limit:
                    keep, extra = waits[:limit], waits[limit:]
                    for k in range(0, len(extra), cap):
                        ev = mybir.InstEventSemaphore(
                            name=f"WSPLIT-{nc.next_id()}", ins=[], outs=[])
                        ev.engine = ins.engine
                        ev.sync_info = mybir.SyncInfo(
                            on_wait=extra[k:k + cap], on_update=[])
                        out.append(ev)
                        n_split += 1
                    si.on_wait = keep
                    dirty = True
                out.append(ins)
            if dirty:
                blk.instructions = out
    return n_split


def _build():
    if 'nc' in _CACHE:
        return _CACHE['nc']
    _patch_tile_drain()

    nc = bass.Bass()

    # ---- dram parameters ----
    x_d = nc.dram_tensor("x", [NTOK, DIM], F32, kind="ExternalInput")
    xbf_d = nc.dram_tensor("xbf", [NTOK, DIM], BF16, kind="ExternalInput")
    # per window-GROUP (3 per band: pair, pair, single) with the windows'
    # key columns pre-interleaved so one 3D DMA loads the whole group
    eq_d = nc.dram_tensor("eq", [15, NH, EB, 2 * N], F8, kind="ExternalInput")
    ek_d = nc.dram_tensor("ek", [15, NH, EB, 2 * N], F8, kind="ExternalInput")
    wqk_d = nc.dram_tensor("wqk", [DIM, 2 * DIM], F8, kind="ExternalInput")
    wv_d = nc.dram_tensor("wv", [DIM, DIM], F8, kind="ExternalInput")
    bqk_d = nc.dram_tensor("bqk", [12, 128], F32, kind="ExternalInput")
    vb_d = nc.dram_tensor("vb", [1, DIM], F32, kind="ExternalInput")
    wp_d = nc.dram_tensor("wp", [DIM, DIM], F8, kind="ExternalInput")
    pb_d = nc.dram_tensor("pb", [1, DIM], F32, kind="ExternalInput")
    w1_d = nc.dram_tensor("w1", [DIM, 4 * DIM], F8, kind="ExternalInput")
    b1_d = nc.dram_tensor("b1", [24, 128], F32, kind="ExternalInput")
    w2_d = nc.dram_tensor("w2", [4 * DIM, DIM], BF16, kind="ExternalInput")
    b2_d = nc.dram_tensor("b2", [1, DIM], F32, kind="ExternalInput")
    y_d = nc.dram_tensor("y", [NTOK, DIM], F32, kind="ExternalOutput")

    dbg = os.environ.get('KERNEL_DEBUG') == '1'
    skind = dict(kind="ExternalOutput") if dbg else {}
    # padded 70x70 images for the LN1 output and the attention output:
    # pad region of xn1 is zeroed once so every gather/scatter is composite
    xn1_d = nc.dram_tensor("xn1", [HP * HP, DIM], F8)
    at_d = nc.dram_tensor("attn", [HP * HP, DIM], BF16, **skind)

    xbf_t32 = xbf_d.rearrange("(a p) d -> a p d", p=128)  # 32 token tiles
    x_pt = x_d.rearrange("(a p) d -> p a d", p=128)       # grouped loads
    xn1_img = xn1_d.rearrange("(r c) d -> r c d", c=HP)
    at_img = at_d.rearrange("(r c) d -> r c d", c=HP)
    y_pt = y_d.rearrange("(a p) d -> p a d", p=128)

    inv_w = 1.0 / WSCALE

    with tile.TileContext(nc, pool_alloc_mode='queue') as tc:
        with tc.tile_pool(name="cW", bufs=1) as pcw, \
             tc.tile_pool(name="lnA", bufs=2) as pa, \
             tc.tile_pool(name="xtP", bufs=7) as pxt, \
             tc.tile_pool(name="xwP", bufs=2) as pxw, \
             tc.tile_pool(name="xwtP", bufs=2) as pxwt, \
             tc.tile_pool(name="qkP", bufs=1) as pqk, \
             tc.tile_pool(name="vP", bufs=2) as pv, \
             tc.tile_pool(name="hdP", bufs=4) as phd, \
             tc.tile_pool(name="owP", bufs=2) as pow_, \
             tc.tile_pool(name="gC", bufs=2) as pg, \
             tc.tile_pool(name="agC", bufs=1) as pag, \
             tc.tile_pool(name="yC", bufs=1) as py, \
             tc.tile_pool(name="hC", bufs=1) as ph, \
             tc.tile_pool(name="gX", bufs=1) as pgx, \
             tc.tile_pool(name="psB", bufs=6, space="PSUM") as psb, \
             tc.tile_pool(name="ptB", bufs=2, space="PSUM") as ptb:

            # ---- persistent weights / consts (scalar HWDGE ring: keeps the
            # sync ring free for steady-state x/xn1/eq/ek traffic) ----
            w1_sb = pcw.tile([128, 6, 4 * DIM], F8)
            nc.scalar.dma_start(out=w1_sb[:], in_=w1_d.rearrange("(k p) n -> p k n", p=128))
            b1_sb = pcw.tile([128, 24], F32)
            nc.scalar.dma_start(out=b1_sb[:], in_=b1_d.rearrange("a p -> p a"))
            if not _CACHE.get('b2_zero'):
                b2_sb = pcw.tile([128, DIM], F32)
                nc.gpsimd.dma_start(out=b2_sb[:], in_=b2_d[0:1, :].to_broadcast((128, DIM)))
            w2_sb = pcw.tile([128, 24, DIM], BF16)
            nc.scalar.dma_start(out=w2_sb[:], in_=w2_d.rearrange("(k p) n -> p k n", p=128))
            eps_t = pcw.tile([128, 1], F32)
            nc.vector.memset(eps_t[:], EPS)
            ident = pcw.tile([128, 128], F8)
            make_identity(nc, ident[:])
            wqk_sb = pcw.tile([128, 6, 2 * DIM], F8)
            nc.scalar.dma_start(out=wqk_sb[:], in_=wqk_d.rearrange("(k p) n -> p k n", p=128))
            wv_sb = pcw.tile([128, 6, DIM], F8)
            nc.scalar.dma_start(out=wv_sb[:], in_=wv_d.rearrange("(k p) n -> p k n", p=128))
            wp_sb = pcw.tile([128, 6, DIM], F8)
            nc.scalar.dma_start(out=wp_sb[:], in_=wp_d.rearrange("(k p) n -> p k n", p=128))
            bqk_sb = pcw.tile([128, 12], F32)
            nc.scalar.dma_start(out=bqk_sb[:], in_=bqk_d.rearrange("a p -> p a"))
            if not _CACHE.get('vb_zero'):
                vb_sb = pcw.tile([128, DIM], F32)
                nc.gpsimd.dma_start(out=vb_sb[:], in_=vb_d[0:1, :].to_broadcast((128, DIM)))
            if not _CACHE.get('pb_zero'):
                pb_sb = pcw.tile([128, DIM], F32)
                nc.gpsimd.dma_start(out=pb_sb[:], in_=pb_d[0:1, :].to_broadcast((128, DIM)))

            # zero the xn1 pad region once (right pad cols 64:70 of rows 0:64,
            # bottom rows 64:70) so edge-window gathers read exact zeros
            zt = pcw.tile([128, DIM], F8)
            nc.vector.memset(zt[:], 0.0)
            for i in range(4):
                nc.gpsimd.dma_start(out=xn1_img[16 * i:16 * i + 16, HH:HP, :],
                                    in_=zt[0:96, :])
            for r in range(HH, HP):
                nc.gpsimd.dma_start(out=xn1_img[r:r + 1, 0:HP, :], in_=zt[0:HP, :])

            sig_gelu = os.environ.get('KERNEL_GELU') == 'sig'

            def emit_ln1_band(band):
                """LN1 for this band's token tiles; batched sqrt for the band."""
                band_tiles = [7, 7, 7, 7, 4]
                nbt = band_tiles[band]
                xts = []
                mvb = pa.tile([128, 2, 7], F32, tag="mvb")
                for bt in range(nbt):
                    t = band * 7 + bt
                    xt = pxt.tile([128, DIM], BF16, tag="xt")
                    nc.sync.dma_start(out=xt[:], in_=xbf_t32[t])
                    st = pa.tile([128, 2, 6], F32, tag="st")
                    for s in range(2):
                        nc.vector.bn_stats(out=st[:, s, :], in_=xt[:, s * 384:(s + 1) * 384])
                    nc.vector.bn_aggr(out=mvb[:, :, bt], in_=st[:])
                    xts.append(xt)
                sdb = pa.tile([128, 7], F32, tag="sdb")
                nc.scalar.activation(out=sdb[:, 0:nbt], in_=mvb[:, 1, 0:nbt],
                                     func=mybir.ActivationFunctionType.Sqrt,
                                     bias=eps_t[:], scale=1.0)
                rsd = pa.tile([128, 7], F32, tag="rsd")
                nc.vector.reciprocal(out=rsd[:, 0:nbt], in_=sdb[:, 0:nbt])
                for bt in range(nbt):
                    xn = pa.tile([128, DIM], F8, tag="xn")
                    nc.vector.tensor_scalar(out=xn[:], in0=xts[bt][:],
                                            scalar1=mvb[:, 0, bt:bt + 1],
                                            scalar2=rsd[:, bt:bt + 1],
                                            op0=mybir.AluOpType.subtract,
                                            op1=mybir.AluOpType.mult)
                    r0 = band * WS + 2 * bt
                    nc.sync.dma_start(out=xn1_img[r0:r0 + 2, 0:HH, :], in_=xn[:])

            def emit_window_group(wins):
                """One group (pair or lone window): qkv, per-window V + pipelined
                heads + proj + scatter."""
                nwin = len(wins)
                wfree = N * nwin
                FPAD = 400 if nwin == 2 else 208   # fp8 Ko-step must be %16
                xwtb = pxwt.tile([128, 6, FPAD], F8, tag="xwtb")
                # qk2: slots 0:12 = per-head [q(64); Eq(32)], 12:24 = [k; Ek]
                qk2 = pqk.tile([128, 24, FPAD], F8, tag="qk2")
                att = pxwt.tile([128, 6, FPAD], F8, tag="att")

                # rel-pos bias operands into the qk2 bias rows 64:96
                # (one 3D DMA per side covers the whole group)
                gid = (wins[0] // NWS) * 3 + {0: 0, 2: 1, 4: 2}[wins[0] % NWS]
                nc.sync.dma_start(out=qk2[64:96, 0:12, 0:wfree],
                                  in_=eq_d[gid, :, :, 0:wfree].rearrange("h r i -> r h i"))
                nc.sync.dma_start(out=qk2[64:96, 12:24, 0:wfree],
                                  in_=ek_d[gid, :, :, 0:wfree].rearrange("h r i -> r h i"))

                # gather + transpose into xwtb (always composite: xn1 is padded)
                for ww_i, w in enumerate(wins):
                    woff = ww_i * N
                    wr, wc = _win_rc(w)
                    xw = pxw.tile([128, 2, DIM], F8, tag="xw")
                    for c in range(2):
                        nc.gpsimd.dma_start(
                            out=xw[0:CH, c, :],
                            in_=xn1_img[wr * WS + c * 7:wr * WS + c * 7 + 7,
                                        wc * WS:wc * WS + WS, :])
                    # transpose via regular identity matmul: out = xw_slice.T @ I
                    for c, coff in ((0, 0), (1, CH)):
                        for j in range(6):
                            pt = ptb.tile([128, 128], F32, tag="pt")
                            nc.tensor.matmul(
                                pt[0:128, 0:CH],
                                lhsT=xw[0:CH, c, j * 128:(j + 1) * 128],
                                rhs=ident[0:CH, 0:CH],
                                start=True, stop=True)
                            nc.vector.tensor_copy(
                                out=xwtb[:, j, woff + coff:woff + coff + CH],
                                in_=pt[0:128, 0:CH])

                # qkv^T for the whole group (fp8 DoubleRow over k-tile pairs);
                # evacuation splits each 2-head PSUM block into per-head slots:
                # even half on ACT (aligned), odd half on DVE (partition-shift)
                for oc in range(12):
                    pqm = psb.tile([128, 392], F32, tag="ps")
                    for kp in range(3):
                        nc.tensor.matmul(
                            pqm[:, 0:wfree],
                            lhsT=wqk_sb[:, 2 * kp:2 * kp + 2, oc * 128:(oc + 1) * 128],
                            rhs=xwtb[:, 2 * kp:2 * kp + 2, 0:wfree],
                            perf_mode=DR,
                            start=(kp == 0), stop=(kp == 2))
                    slot = 2 * (oc % 6) + (12 if oc >= 6 else 0)
                    nc.scalar.activation(out=qk2[0:64, slot, 0:wfree],
                                         in_=pqm[0:64, 0:wfree],
                                         func=mybir.ActivationFunctionType.Identity,
                                         bias=bqk_sb[0:64, oc:oc + 1], scale=inv_w)
                    nc.vector.tensor_scalar(out=qk2[0:64, slot + 1, 0:wfree],
                                            in0=pqm[64:128, 0:wfree],
                                            scalar1=inv_w,
                                            scalar2=bqk_sb[64:128, oc:oc + 1],
                                            op0=mybir.AluOpType.mult,
                                            op1=mybir.AluOpType.add)

                for ww_i, w in enumerate(wins):
                    woff = ww_i * N
                    # V (fp8): all heads + 64 ones columns for the denominator
                    va = pv.tile([128, 2, DIM + 64], F8, tag="va")
                    for c, coff in ((0, 0), (1, CH)):
                        nc.gpsimd.memset(va[0:CH, c, DIM:DIM + 64], 1.0)
                        pv0 = psb.tile([128, 384], F32, tag="ps")
                        pv1 = psb.tile([128, 384], F32, tag="ps")
                        for kp in range(3):
                            nc.tensor.matmul(
                                pv0[0:CH, :],
                                lhsT=xwtb[:, 2 * kp:2 * kp + 2,
                                          woff + coff:woff + coff + CH],
                                rhs=wv_sb[:, 2 * kp:2 * kp + 2, 0:384],
                                perf_mode=DR,
                                start=(kp == 0), stop=(kp == 2))
                            nc.tensor.matmul(
                                pv1[0:CH, :],
                                lhsT=xwtb[:, 2 * kp:2 * kp + 2,
                                          woff + coff:woff + coff + CH],
                                rhs=wv_sb[:, 2 * kp:2 * kp + 2, 384:768],
                                perf_mode=DR,
                                start=(kp == 0), stop=(kp == 2))
                        for half, pvm in ((0, pv0), (1, pv1)):
                            if _CACHE.get('vb_zero'):
                                nc.vector.tensor_scalar(
                                    out=va[0:CH, c, half * 384:(half + 1) * 384],
                                    in0=pvm[0:CH, :], scalar1=inv_w, scalar2=None,
                                    op0=mybir.AluOpType.mult)
                            else:
                                nc.vector.scalar_tensor_tensor(
                                    out=va[0:CH, c, half * 384:(half + 1) * 384],
                                    in0=pvm[0:CH, :],
                                    scalar=inv_w,
                                    in1=vb_sb[0:CH, half * 384:(half + 1) * 384],
                                    op0=mybir.AluOpType.mult,
                                    op1=mybir.AluOpType.add)

                    # heads: merged QK+bias for pair p, then PV/normalize p-1
                    pTs = {}
                    psos = {}

                    def emit_qk(h):
                        pss = psb.tile([128, 2 * N], F32, tag="ps")
                        for c in range(2):
                            nc.tensor.matmul(
                                pss[0:CH, c * N:(c + 1) * N],
                                lhsT=qk2[0:96, 12 + h,
                                         woff + c * CH:woff + c * CH + CH],
                                rhs=qk2[0:96, h, woff:woff + N],
                                start=True, stop=True)
                        pT = phd.tile([128, 2, 208], F8, tag="pT")
                        nc.scalar.activation(out=pT[0:CH, :, 0:N], in_=pss[0:CH, 0:2 * N],
                                             func=mybir.ActivationFunctionType.Exp)
                        pTs[h] = pT

                    def emit_pv(p):
                        pso = psb.tile([128, 2 * N], F32, tag="ps")
                        for h in (2 * p, 2 * p + 1):
                            b0 = (h % 2) * 64
                            pT = pTs.pop(h)
                            if b0 == 0:
                                # DoubleRow folds both key-chunks into one pass
                                nc.tensor.matmul(pso[0:64, 0:N],
                                                 lhsT=va[0:CH, 0:2, h * 64:(h + 1) * 64],
                                                 rhs=pT[0:CH, 0:2, 0:N],
                                                 perf_mode=DR, start=True, stop=True,
                                                 skip_group_check=True)
                                nc.tensor.matmul(pso[0:64, N:2 * N],
                                                 lhsT=va[0:CH, 0:2, DIM:DIM + 64],
                                                 rhs=pT[0:CH, 0:2, 0:N],
                                                 perf_mode=DR, start=True, stop=True,
                                                 skip_group_check=True)
                            else:
                                # walrus rejects DoubleRow + col-offset
                                # tile_position; plain fp8 per chunk instead
                                for c in range(2):
                                    nc.tensor.matmul(pso[64:128, 0:N],
                                                     lhsT=va[0:CH, c, h * 64:(h + 1) * 64],
                                                     rhs=pT[0:CH, c, 0:N],
                                                     start=(c == 0), stop=(c == 1),
                                                     skip_group_check=True)
                                    nc.tensor.matmul(pso[64:128, N:2 * N],
                                                     lhsT=va[0:CH, c, DIM:DIM + 64],
                                                     rhs=pT[0:CH, c, 0:N],
                                                     start=(c == 0), stop=(c == 1),
                                                     skip_group_check=True)
                        psos[p] = pso

                    def emit_norm(p):
                        pso = psos.pop(p)
                        rb = phd.tile([128, N], F32, tag="rb")
                        nc.vector.reciprocal(out=rb[:], in_=pso[:, N:2 * N])
                        nc.vector.tensor_mul(out=att[:, p, woff:woff + N],
                                             in0=pso[:, 0:N], in1=rb[:])

                    for p in range(6):
                        emit_qk(2 * p)
                        emit_qk(2 * p + 1)
                        if p >= 1:
                            emit_pv(p - 1)
                            emit_norm(p - 1)
                    emit_pv(5)
                    emit_norm(5)

                    # proj (fp8 DoubleRow) -> ow, then unpartition scatter
                    ow = pow_.tile([128, 2, DIM], BF16, tag="ow")
                    for c, coff in ((0, 0), (1, CH)):
                        pp0 = psb.tile([128, 384], F32, tag="ps")
                        pp1 = psb.tile([128, 384], F32, tag="ps")
                        for kp in range(3):
                            nc.tensor.matmul(
                                pp0[0:CH, :],
                                lhsT=att[:, 2 * kp:2 * kp + 2,
                                         woff + coff:woff + coff + CH],
                                rhs=wp_sb[:, 2 * kp:2 * kp + 2, 0:384],
                                perf_mode=DR,
                                start=(kp == 0), stop=(kp == 2))
                            nc.tensor.matmul(
                                pp1[0:CH, :],
                                lhsT=att[:, 2 * kp:2 * kp + 2,
                                         woff + coff:woff + coff + CH],
                                rhs=wp_sb[:, 2 * kp:2 * kp + 2, 384:768],
                                perf_mode=DR,
                                start=(kp == 0), stop=(kp == 2))
                        for half, psp in ((0, pp0), (1, pp1)):
                            if _CACHE.get('pb_zero'):
                                nc.scalar.activation(
                                    out=ow[0:CH, c, half * 384:(half + 1) * 384],
                                    in_=psp[0:CH, :],
                                    func=mybir.ActivationFunctionType.Copy,
                                    bias=0.0, scale=inv_w)
                            else:
                                nc.vector.scalar_tensor_tensor(
                                    out=ow[0:CH, c, half * 384:(half + 1) * 384],
                                    in0=psp[0:CH, :], scalar=inv_w,
                                    in1=pb_sb[0:CH, half * 384:(half + 1) * 384],
                                    op0=mybir.AluOpType.mult,
                                    op1=mybir.AluOpType.add)
                    wr, wc = _win_rc(w)
                    for c in range(2):
                        nc.gpsimd.dma_start(
                            out=at_img[wr * WS + c * 7:wr * WS + c * 7 + 7,
                                       wc * WS:wc * WS + WS, :],
                            in_=ow[0:CH, c, :])

            def emit_c_group(g):
                """Phase C for token tiles 4g..4g+3 (512 tokens = 8 image
                rows): residual, LN2, MLP, out.  4-tile grouping halves the
                gelu / bn / DMA fixed costs vs per-2-tile groups."""
                xg = pg.tile([128, 4, DIM], F32, tag="xg")
                ag = pag.tile([128, 4, DIM], BF16, tag="ag")
                nc.scalar.dma_start(out=xg[:], in_=x_pt[:, 4 * g:4 * g + 4, :])
                for a in range(4):
                    r0 = 8 * g + 2 * a
                    nc.scalar.dma_start(out=ag[:, a, :],
                                        in_=at_img[r0:r0 + 2, 0:HH, :])
                # x2 = x + attn (in place into xg)
                nc.vector.tensor_add(out=xg[:, :, :], in0=xg[:, :, :], in1=ag[:, :, :])
                xn2t = pgx.tile([128, 6, 512], F8, tag="xn2t")
                mvc = pg.tile([128, 2, 4], F32, tag="mvc")
                for s in range(4):
                    st = pg.tile([128, 2, 6], F32, tag="stC")
                    for sub in range(2):
                        nc.vector.bn_stats(out=st[:, sub, :],
                                           in_=xg[:, s, sub * 384:(sub + 1) * 384])
                    nc.vector.bn_aggr(out=mvc[:, :, s], in_=st[:])
                sdc = pg.tile([128, 4], F32, tag="sdC")
                nc.scalar.activation(out=sdc[:], in_=mvc[:, 1, :],
                                     func=mybir.ActivationFunctionType.Sqrt,
                                     bias=eps_t[:], scale=1.0)
                rsc = pg.tile([128, 4], F32, tag="rsC")
                nc.vector.reciprocal(out=rsc[:], in_=sdc[:])
                for s in range(4):
                    xn2b = pg.tile([128, DIM], F8, tag="xn2b")
                    nc.vector.tensor_scalar(out=xn2b[:, :], in0=xg[:, s, :],
                                            scalar1=mvc[:, 0, s:s + 1],
                                            scalar2=rsc[:, s:s + 1],
                                            op0=mybir.AluOpType.subtract,
                                            op1=mybir.AluOpType.mult)
                    if not _CACHE.get('b2_zero'):
                        nc.vector.tensor_add(out=xg[:, s, :], in0=xg[:, s, :],
                                             in1=b2_sb[:])
                    for j in range(6):
                        pt = ptb.tile([128, 128], F32, tag="pt")
                        nc.tensor.matmul(pt[:, :],
                                         lhsT=xn2b[:, j * 128:(j + 1) * 128],
                                         rhs=ident[:, :], start=True, stop=True)
                        nc.vector.tensor_copy(out=xn2t[:, j, s * 128:(s + 1) * 128],
                                              in_=pt[:, :])
                h1t = ph.tile([128, 24, 512], BF16, tag="h1t")
                for oc in range(24):
                    psh = psb.tile([128, 512], F32, tag="ps")
                    for kp in range(3):
                        nc.tensor.matmul(
                            psh[:, :],
                            lhsT=w1_sb[:, 2 * kp:2 * kp + 2, oc * 128:(oc + 1) * 128],
                            rhs=xn2t[:, 2 * kp:2 * kp + 2, :],
                            perf_mode=DR,
                            start=(kp == 0), stop=(kp == 2))
                    if sig_gelu:
                        # CoreSim lacks Gelu; x*sigmoid(1.702x) validates shapes
                        hpre = pg.tile([128, 512], BF16, tag="hpre")
                        nc.scalar.activation(out=hpre[:], in_=psh[:, :],
                                             func=mybir.ActivationFunctionType.Identity,
                                             bias=b1_sb[:, oc:oc + 1], scale=inv_w)
                        sg = pg.tile([128, 512], BF16, tag="sg")
                        nc.scalar.activation(out=sg[:], in_=hpre[:],
                                             func=mybir.ActivationFunctionType.Sigmoid,
                                             bias=0.0, scale=1.702)
                        nc.vector.tensor_mul(out=h1t[:, oc, :], in0=hpre[:], in1=sg[:])
                    else:
                        nc.scalar.activation(out=h1t[:, oc, :], in_=psh[:, :],
                                             func=mybir.ActivationFunctionType.Gelu,
                                             bias=b1_sb[:, oc:oc + 1], scale=inv_w)
                for sp in range(2):
                    yo = py.tile([128, 2, DIM], F32, tag="yo")
                    for ss in range(2):
                        s = 2 * sp + ss
                        pf0 = psb.tile([128, 384], F32, tag="ps")
                        pf1 = psb.tile([128, 384], F32, tag="ps")
                        for kt in range(24):
                            nc.tensor.matmul(
                                pf0[:, :],
                                lhsT=h1t[:, kt, s * 128:(s + 1) * 128],
                                rhs=w2_sb[:, kt, 0:384],
                                start=(kt == 0), stop=(kt == 23))
                            nc.tensor.matmul(
                                pf1[:, :],
                                lhsT=h1t[:, kt, s * 128:(s + 1) * 128],
                                rhs=w2_sb[:, kt, 384:768],
                                start=(kt == 0), stop=(kt == 23))
                        for half, psf in ((0, pf0), (1, pf1)):
                            nc.vector.tensor_add(
                                out=yo[:, ss, half * 384:(half + 1) * 384],
                                in0=psf[:, :],
                                in1=xg[:, s, half * 384:(half + 1) * 384])
                    nc.scalar.dma_start(out=y_pt[:, 4 * g + 2 * sp:4 * g + 2 * sp + 2, :],
                                        in_=yo[:])

            # phase C double-group G covers image rows 8G..8G+8; ready once
            # the band containing its last row is done.  Groups are POPPED
            # one band later (after that band's last window group) so band
            # boundaries always have PE-ready work.
            c_ready = {0: [0], 1: [1, 2], 2: [3, 4], 3: [5, 6], 4: [7]}
            pending = []

            emit_ln1_band(0)
            for band in range(5):
                w0 = band * NWS
                emit_window_group((w0, w0 + 1))
                # overlap next band's LN1 (DVE/DMA) with this band's windows
                if band < 4:
                    emit_ln1_band(band + 1)
                emit_window_group((w0 + 2, w0 + 3))
                emit_window_group((w0 + 4,))
                while pending:
                    emit_c_group(pending.pop(0))
                pending.extend(c_ready[band])
            for g in pending:
                emit_c_group(g)

    if os.environ.get('KERNEL_NOLDDEDUP') != '1':
        _dedup_ldweights(nc)
    if os.environ.get('KERNEL_SIM') != '1':
        _split_waits(nc)
    _CACHE['nc'] = nc
    return nc


def _host_prep(inputs):
    """Fold LN affines into matmul weights, build rel-pos operands."""
    f32 = np.float32
    x = np.asarray(inputs['x'], f32)
    q_idx = np.asarray(inputs['q_idx']).astype(np.int64)
    k_idx = np.asarray(inputs['k_idx']).astype(np.int64)
    ln1_w = np.asarray(inputs['ln1_w'], f32); ln1_b = np.asarray(inputs['ln1_b'], f32)
    ln2_w = np.asarray(inputs['ln2_w'], f32); ln2_b = np.asarray(inputs['ln2_b'], f32)
    qkv_w = np.asarray(inputs['qkv_w'], f32); qkv_b = np.asarray(inputs['qkv_b'], f32)
    proj_w = np.asarray(inputs['proj_w'], f32); proj_b = np.asarray(inputs['proj_b'], f32)
    mlp_w1 = np.asarray(inputs['mlp_w1'], f32); mlp_b1 = np.asarray(inputs['mlp_b1'], f32)
    mlp_w2 = np.asarray(inputs['mlp_w2'], f32); mlp_b2 = np.asarray(inputs['mlp_b2'], f32)
    rel_h = np.asarray(inputs['rel_h'], f32); rel_w = np.asarray(inputs['rel_w'], f32)

    scale = HD ** -0.5
    Wqkv = ln1_w[:, None] * qkv_w
    bqkv = ln1_b @ qkv_w + qkv_b
    Wqkv = Wqkv.copy(); bqkv = bqkv.copy()
    Wqkv[:, :DIM] *= scale
    bqkv[:DIM] *= scale
    W1 = ln2_w[:, None] * mlp_w1
    b1 = ln2_b @ mlp_w1 + mlp_b1

    coords = np.arange(WS)[:, None] - np.arange(WS)[None, :] + (WS - 1)
    Sh = rel_h[coords].sum(-1).astype(f32)
    Sw = rel_w[coords].sum(-1).astype(f32)

    qr, qc = q_idx // WS, q_idx % WS
    kr, kc = k_idx // WS, k_idx % WS
    nb = q_idx.shape[0]
    Eq = np.zeros((nb, EB, N), f32)
    Eq[:, 0:WS, :] = np.take(Sh, qr, axis=0).transpose(0, 2, 1)
    Eq[:, WS:2 * WS, :] = np.take(Sw, qc, axis=0).transpose(0, 2, 1)
    Ek = np.zeros((nb, EB, N), f32)
    bi = np.arange(nb)[:, None]
    ar = np.arange(N)[None, :]
    Ek[bi, kr, ar] = 1.0
    Ek[bi, WS + kc, ar] = 1.0

    bf = ml_dtypes.bfloat16
    f8 = ml_dtypes.float8_e4m3fn
    shared = {
        "wqk": np.ascontiguousarray(Wqkv[:, :2 * DIM] * WSCALE).astype(f8),
        "wv": np.ascontiguousarray(Wqkv[:, 2 * DIM:] * WSCALE).astype(f8),
        "bqk": np.ascontiguousarray(bqkv[:2 * DIM].reshape(12, 128)),
        "vb": np.ascontiguousarray(bqkv[2 * DIM:].reshape(1, DIM)),
        "wp": np.ascontiguousarray(proj_w * WSCALE).astype(f8),
        "pb": proj_b.reshape(1, DIM).copy(),
        "w1": np.ascontiguousarray(W1 * WSCALE).astype(f8),
        "b1": np.ascontiguousarray(b1.reshape(24, 128)),
        "w2": mlp_w2.astype(bf),
        "b2": mlp_b2.reshape(1, DIM).copy(),
    }
    Eq = Eq.astype(f8).reshape(B, NW, NH, EB, N)
    Ek = Ek.astype(f8).reshape(B, NW, NH, EB, N)
    # regroup per window-group (pair, pair, single per band), windows'
    # key columns contiguous on the last axis
    EqG = np.zeros((B, 15, NH, EB, 2 * N), f8)
    EkG = np.zeros((B, 15, NH, EB, 2 * N), f8)
    for band in range(5):
        for gi, ws_ in enumerate(((0, 1), (2, 3), (4,))):
            g = band * 3 + gi
            for wi, wo in enumerate(ws_):
                w = band * NWS + wo
                EqG[:, g, :, :, wi * N:(wi + 1) * N] = Eq[:, w]
                EkG[:, g, :, :, wi * N:(wi + 1) * N] = Ek[:, w]
    in_maps = []
    for b in range(B):
        m = dict(shared)
        m["x"] = np.ascontiguousarray(x[b].reshape(NTOK, DIM))
        m["xbf"] = np.ascontiguousarray(x[b].reshape(NTOK, DIM)).astype(bf)
        m["eq"] = np.ascontiguousarray(EqG[b])
        m["ek"] = np.ascontiguousarray(EkG[b])
        in_maps.append(m)
    return in_maps


def kernel(**inputs):
    in_maps = _host_prep(inputs)
    if 'nc' not in _CACHE:
        _CACHE['pb_zero'] = not np.any(np.asarray(in_maps[0]['pb'], np.float32))
        _CACHE['b2_zero'] = not np.any(np.asarray(in_maps[0]['b2'], np.float32))
    nc = _build()
    trace = os.environ.get('KERNEL_TRACE') == '1'
    if trace:
        _install_ntff_hook()
    res = run_bass_kernel_spmd(nc, in_maps, list(range(B)), trace=trace)
    if trace and res.exec_time_ns is not None:
        print(f"HW exec time: {res.exec_time_ns} ns")
        _CACHE['exec_time_ns'] = res.exec_time_ns
    _CACHE['last_results'] = res
    out = np.stack([np.asarray(res.results[b]["y"]).reshape(HH, WW, DIM)
                    for b in range(B)])
    return out.astype(np.float32)


# revision 28
# speedup vs baseline: 1.0238x; 1.0238x over previous
"""Trainium2 Bass kernel for nn_Block_72138270704025 (windowed sparse attention
block: LN1 -> window partition -> MHA with decomposed rel-pos bias gathered by
q_idx/k_idx -> window unpartition -> residual -> LN2 -> MLP(gelu) -> residual).

Sharding: data-parallel over batch B=8, one batch element per NeuronCore; all
weights replicated.  Host folds LN affine params into the adjacent matmul
weights, precomputes the rel-pos tables Sh/Sw, and turns the per-(window,head)
index gathers into small per-window fp8 operands so the bias folds into the
logits matmul.

v3 changes over v2:
- Merged QK+bias logits: per head the contraction is the 96-row concat
  [k_h(64); Ek_h(28+4 zero pad)] x [q_h(64); Eq_h(32)], so ONE matmul per
  (head, key-chunk) produces logits+bias (v2 used two).  Halves the logits
  matmul and ldweights count.  Odd heads' q/k are evacuated from PSUM rows
  64:128 to SBUF rows 0:64 with a partition-shifted DVE op (64-channel DVE
  ops may read any source partition window); even heads evacuate on ACT.
- Phase C (residual+LN2+MLP) groups deferred one band and interleaved
  BETWEEN window groups, so band boundaries always have ready PE work and
  the serial scatter->load->LN2 chain of a fresh c-group is hidden.
- xn1 and attn intermediates stored as zero-padded 70x70 images in DRAM:
  every window gather/scatter is 2 composite 3D DMAs (v2 used per-row DMAs
  plus memsets for the 9 edge windows), shortening the GpSimd DMA queue.

Carried over from v2: fp8e4m3 DoubleRow matmuls for qkv/v/proj/fc1 and PV
(weights x32 host scale), softmax reciprocal per head-pair, head software
pipelining, transposes as identity matmuls, batched LN sqrt, bf16 fc2.
"""
import os
import sys

for _p in ('/opt/trn_rl_repo', '/root/.axon_site/_ro/trn_rl_repo'):
    if os.path.isdir(_p) and _p not in sys.path:
        sys.path.append(_p)

import numpy as np
import ml_dtypes

import concourse.bass as bass
import concourse.tile as tile
from concourse import mybir
from concourse.bass_utils import run_bass_kernel_spmd
from concourse.tile import ScopedClock
from concourse.masks import make_identity

# ---- problem constants (hardcoded per contest rules) ----
B = 8
HH = 64
WW = 64
DIM = 768
NH = 12
WS = 14
HD = 64
N = 196            # tokens per window
NWS = 5            # window grid side
NW = 25            # windows per image
EPS = 1e-5
NTOK = HH * WW     # 4096 tokens per core
CH = 98            # window token chunk: 7 rows of 14 (196 = 2x98)
WSCALE = 32.0      # host fp8 weight scale (descaled at PSUM evacuation)
HP = 70            # padded image side (5 windows x 14)
EB = 32            # bias contraction rows (28 used + 4 zero pad)

F32 = mybir.dt.float32
BF16 = mybir.dt.bfloat16
F8 = mybir.dt.float8e4
DR = mybir.MatmulPerfMode.DoubleRow


def _patch_tile_drain():
    """Walrus CoreV3 codegen rejects a Drain carrying multiple sem waits
    ("Too many sync wait commands").  Emit explicit wait_ge instructions
    before the kernel-tail drain instead."""
    if getattr(tile.TileContext, '_drain_patched', False):
        return

    def _drain_and_barrier(self, tick_clock, wait_clock):
        nc = self.nc
        dummy = nc.sync.nop(nofuse=True)
        wait_clock.add_sem_waits(dummy.ins, ScopedClock({None: tick_clock.global_clock}))
        waits = list(dummy.ins.sync_info.on_wait or [])
        dummy.ins.sync_info.on_wait = []
        assert self.sems is not None
        by_id = {}
        for h in self.sems.allocated().values():
            by_id[getattr(h, 'id', None)] = h
            by_id[getattr(h, 'name', None)] = h
        for w in waits:
            h = by_id.get(w.id) or by_id.get(w.ant_name)
            assert h is not None, (w.id, w.ant_name)
            nc.sync.wait_ge(h, w.wait_value)
        nc.sync.drain()
        nc.all_engine_barrier()
        popped = nc._tile_sem_poison_stack.pop()
        assert popped is self._sem_poison
        nc.clear_and_free_semaphores(list(self.sems.allocated().values()))
        nc.all_engine_barrier()

    tile.TileContext._drain_and_barrier = _drain_and_barrier
    tile.TileContext._drain_patched = True


def _install_ntff_hook():
    """Recreate the missing antenv.axon_hooks module so trace=True can reach
    the axon NTFF profiler (used only when KERNEL_TRACE=1)."""
    try:
        import types
        import antenv
        if 'antenv.axon_hooks' in sys.modules:
            return True
        mod = types.ModuleType('antenv.axon_hooks')
        mod._hook = None
        mod.set_axon_ntff_profile_hook = lambda h: setattr(mod, '_hook', h)
        mod.get_axon_ntff_profile_hook = lambda: mod._hook
        sys.modules['antenv.axon_hooks'] = mod
        antenv.axon_hooks = mod
        from trn_agent_boot.trn_boot import _ntff_profile_via_ctypes
        mod._hook = _ntff_profile_via_ctypes('/opt/axon/libaxon_pjrt.so')
        return mod._hook is not None
    except Exception:
        return False


# window geometry helpers
def _win_rc(w):
    return w // NWS, w % NWS


_CACHE = {}


def _dedup_ldweights(nc):
    """Tile lowers each matmul to Ldweights+Matmult.  Back-to-back matmuls
    that share a stationary operand reload identical weights; drop the
    redundant Ldweights (keeping its sem waits / updates on a zero-cost
    EventSemaphore)."""
    ndrop = 0
    for fn in nc.m.functions:
        for blk in fn.blocks:
            insts = blk.instructions
            out = []
            prev_key = None
            dirty = False
            for ins in insts:
                if ins.engine != mybir.EngineType.PE:
                    out.append(ins)
                    continue
                if ins.opcode == 'Ldweights':
                    a = ins.ins[0]
                    key = (str(getattr(a, 'memory_location', None)),
                           getattr(a, 'offset', None), str(getattr(a, 'ap', None)),
                           str(getattr(ins, 'is_transpose', None)),
                           str(getattr(ins, 'perf_mode', None)))
                    si = ins.sync_info
                    has_sync = si and (si.on_wait or si.on_update)
                    if key == prev_key:
                        ndrop += 1
                        dirty = True
                        if has_sync:
                            ev = mybir.InstEventSemaphore(
                                name=f"LDDROP-{nc.next_id()}", ins=[], outs=[])
                            ev.engine = ins.engine
                            ev.sync_info = mybir.SyncInfo(
                                on_wait=list(si.on_wait or []),
                                on_update=list(si.on_update or []))
                            out.append(ev)
                        continue
                    prev_key = key
                    out.append(ins)
                elif ins.opcode == 'Matmult' and not getattr(ins, 'is_transpose', False):
                    out.append(ins)
                else:
                    prev_key = None
                    out.append(ins)
            if dirty:
                blk.instructions = out
    return ndrop


def _split_waits(nc, cap=None):
    """Walrus CoreV2/V3 codegen rejects instructions whose sync_info carries
    more waits than the per-opcode ISA ctrl struct holds.  Hoist excess waits
    onto standalone EventSemaphore instructions."""
    if cap is None:
        cap = int(os.environ.get('KERNEL_MAXWAITS', '1'))
    n_split = 0
    for fn in nc.m.functions:
        for blk in fn.blocks:
            insts = blk.instructions
            out = []
            dirty = False
            for ins in insts:
                si = ins.sync_info
                waits = list(si.on_wait) if si and si.on_wait else []
                # InstISA (custom DVE ops) cannot encode sem waits at all
                limit = 0 if ins.opcode == 'ISA' else (
                    1 if ins.opcode in ('Drain',) else cap)
                if len(waits) > limit:
                    keep, extra = waits[:limit], waits[limit:]
                    for k in range(0, len(extra), cap):
                        ev = mybir.InstEventSemaphore(
                            name=f"WSPLIT-{nc.next_id()}", ins=[], outs=[])
                        ev.engine = ins.engine
                        ev.sync_info = mybir.SyncInfo(
                            on_wait=extra[k:k + cap], on_update=[])
                        out.append(ev)
                        n_split += 1
                    si.on_wait = keep
                    dirty = True
                out.append(ins)
            if dirty:
                blk.instructions = out
    return n_split


def _build():
    if 'nc' in _CACHE:
        return _CACHE['nc']
    _patch_tile_drain()

    nc = bass.Bass()

    # ---- dram parameters ----
    x_d = nc.dram_tensor("x", [NTOK, DIM], F32, kind="ExternalInput")
    xbf_d = nc.dram_tensor("xbf", [NTOK, DIM], BF16, kind="ExternalInput")
    # per window-GROUP (3 per band: pair, pair, single) with the windows'
    # key columns pre-interleaved so one 3D DMA loads the whole group
    eq_d = nc.dram_tensor("eq", [15, NH, EB, 2 * N], F8, kind="ExternalInput")
    ek_d = nc.dram_tensor("ek", [15, NH, EB, 2 * N], F8, kind="ExternalInput")
    wqk_d = nc.dram_tensor("wqk", [DIM, 2 * DIM], F8, kind="ExternalInput")
    wv_d = nc.dram_tensor("wv", [DIM, DIM], F8, kind="ExternalInput")
    bqk_d = nc.dram_tensor("bqk", [12, 128], F32, kind="ExternalInput")
    vb_d = nc.dram_tensor("vb", [1, DIM], F32, kind="ExternalInput")
    wp_d = nc.dram_tensor("wp", [DIM, DIM], F8, kind="ExternalInput")
    pb_d = nc.dram_tensor("pb", [1, DIM], F32, kind="ExternalInput")
    w1_d = nc.dram_tensor("w1", [DIM, 4 * DIM], F8, kind="ExternalInput")
    b1_d = nc.dram_tensor("b1", [24, 128], F32, kind="ExternalInput")
    w2_d = nc.dram_tensor("w2", [4 * DIM, DIM], BF16, kind="ExternalInput")
    b2_d = nc.dram_tensor("b2", [1, DIM], F32, kind="ExternalInput")
    y_d = nc.dram_tensor("y", [NTOK, DIM], F32, kind="ExternalOutput")

    dbg = os.environ.get('KERNEL_DEBUG') == '1'
    skind = dict(kind="ExternalOutput") if dbg else {}
    # padded 70x70 images for the LN1 output and the attention output:
    # pad region of xn1 is zeroed once so every gather/scatter is composite
    xn1_d = nc.dram_tensor("xn1", [HP * HP, DIM], F8)
    at_d = nc.dram_tensor("attn", [HP * HP, DIM], BF16, **skind)

    xbf_t32 = xbf_d.rearrange("(a p) d -> a p d", p=128)  # 32 token tiles
    x_pt = x_d.rearrange("(a p) d -> p a d", p=128)       # grouped loads
    xn1_img = xn1_d.rearrange("(r c) d -> r c d", c=HP)
    at_img = at_d.rearrange("(r c) d -> r c d", c=HP)
    y_pt = y_d.rearrange("(a p) d -> p a d", p=128)

    inv_w = 1.0 / WSCALE

    with tile.TileContext(nc, pool_alloc_mode='queue') as tc:
        with tc.tile_pool(name="cW", bufs=1) as pcw, \
             tc.tile_pool(name="lnA", bufs=2) as pa, \
             tc.tile_pool(name="xtP", bufs=7) as pxt, \
             tc.tile_pool(name="xwP", bufs=2) as pxw, \
             tc.tile_pool(name="xwtP", bufs=2) as pxwt, \
             tc.tile_pool(name="qkP", bufs=1) as pqk, \
             tc.tile_pool(name="vP", bufs=2) as pv, \
             tc.tile_pool(name="hdP", bufs=4) as phd, \
             tc.tile_pool(name="owP", bufs=2) as pow_, \
             tc.tile_pool(name="gC", bufs=2) as pg, \
             tc.tile_pool(name="agC", bufs=1) as pag, \
             tc.tile_pool(name="yC", bufs=1) as py, \
             tc.tile_pool(name="hC", bufs=1) as ph, \
             tc.tile_pool(name="gX", bufs=1) as pgx, \
             tc.tile_pool(name="psB", bufs=6, space="PSUM") as psb, \
             tc.tile_pool(name="ptB", bufs=2, space="PSUM") as ptb:

            # ---- persistent weights / consts.  Attention weights go early
            # on the sync HWDGE ring; the bulk MLP weights (only needed from
            # the first phase-C group, ~200us in) load on the gpsimd ring
            # AFTER the first window group's gathers (emit_bulk_weights). ----
            w1_sb = pcw.tile([128, 6, 4 * DIM], F8)
            b1_sb = pcw.tile([128, 24], F32)
            w2_sb = pcw.tile([128, 24, DIM], BF16)
            if not _CACHE.get('b2_zero'):
                b2_sb = pcw.tile([128, DIM], F32)

            def emit_bulk_weights():
                nc.gpsimd.dma_start(out=w1_sb[:],
                                    in_=w1_d.rearrange("(k p) n -> p k n", p=128))
                nc.gpsimd.dma_start(out=b1_sb[:], in_=b1_d.rearrange("a p -> p a"))
                nc.gpsimd.dma_start(out=w2_sb[:],
                                    in_=w2_d.rearrange("(k p) n -> p k n", p=128))
                if not _CACHE.get('b2_zero'):
                    nc.gpsimd.dma_start(out=b2_sb[:],
                                        in_=b2_d[0:1, :].to_broadcast((128, DIM)))

            eps_t = pcw.tile([128, 1], F32)
            nc.vector.memset(eps_t[:], EPS)
            ident = pcw.tile([128, 128], F8)
            make_identity(nc, ident[:])
            wqk_sb = pcw.tile([128, 6, 2 * DIM], F8)
            nc.sync.dma_start(out=wqk_sb[:], in_=wqk_d.rearrange("(k p) n -> p k n", p=128))
            wv_sb = pcw.tile([128, 6, DIM], F8)
            nc.sync.dma_start(out=wv_sb[:], in_=wv_d.rearrange("(k p) n -> p k n", p=128))
            wp_sb = pcw.tile([128, 6, DIM], F8)
            nc.sync.dma_start(out=wp_sb[:], in_=wp_d.rearrange("(k p) n -> p k n", p=128))
            bqk_sb = pcw.tile([128, 12], F32)
            nc.sync.dma_start(out=bqk_sb[:], in_=bqk_d.rearrange("a p -> p a"))
            if not _CACHE.get('vb_zero'):
                vb_sb = pcw.tile([128, DIM], F32)
                nc.gpsimd.dma_start(out=vb_sb[:], in_=vb_d[0:1, :].to_broadcast((128, DIM)))
            if not _CACHE.get('pb_zero'):
                pb_sb = pcw.tile([128, DIM], F32)
                nc.gpsimd.dma_start(out=pb_sb[:], in_=pb_d[0:1, :].to_broadcast((128, DIM)))

            # zero the xn1 pad region once (right pad cols 64:70 of rows 0:64,
            # bottom rows 64:70) so edge-window gathers read exact zeros
            zt = pcw.tile([128, DIM], F8)
            nc.vector.memset(zt[:], 0.0)
            for i in range(4):
                nc.gpsimd.dma_start(out=xn1_img[16 * i:16 * i + 16, HH:HP, :],
                                    in_=zt[0:96, :])
            for r in range(HH, HP):
                nc.gpsimd.dma_start(out=xn1_img[r:r + 1, 0:HP, :], in_=zt[0:HP, :])

            sig_gelu = os.environ.get('KERNEL_GELU') == 'sig'

            def emit_ln1_band(band):
                """LN1 for this band's token tiles; batched sqrt for the band."""
                band_tiles = [7, 7, 7, 7, 4]
                nbt = band_tiles[band]
                xts = []
                mvb = pa.tile([128, 2, 7], F32, tag="mvb")
                for bt in range(nbt):
                    t = band * 7 + bt
                    xt = pxt.tile([128, DIM], BF16, tag="xt")
                    nc.sync.dma_start(out=xt[:], in_=xbf_t32[t])
                    st = pa.tile([128, 2, 6], F32, tag="st")
                    for s in range(2):
                        nc.vector.bn_stats(out=st[:, s, :], in_=xt[:, s * 384:(s + 1) * 384])
                    nc.vector.bn_aggr(out=mvb[:, :, bt], in_=st[:])
                    xts.append(xt)
                sdb = pa.tile([128, 7], F32, tag="sdb")
                nc.scalar.activation(out=sdb[:, 0:nbt], in_=mvb[:, 1, 0:nbt],
                                     func=mybir.ActivationFunctionType.Sqrt,
                                     bias=eps_t[:], scale=1.0)
                rsd = pa.tile([128, 7], F32, tag="rsd")
                nc.vector.reciprocal(out=rsd[:, 0:nbt], in_=sdb[:, 0:nbt])
                for bt in range(nbt):
                    xn = pa.tile([128, DIM], F8, tag="xn")
                    nc.vector.tensor_scalar(out=xn[:], in0=xts[bt][:],
                                            scalar1=mvb[:, 0, bt:bt + 1],
                                            scalar2=rsd[:, bt:bt + 1],
                                            op0=mybir.AluOpType.subtract,
                                            op1=mybir.AluOpType.mult)
                    r0 = band * WS + 2 * bt
                    nc.sync.dma_start(out=xn1_img[r0:r0 + 2, 0:HH, :], in_=xn[:])

            def emit_window_group(wins):
                """One group (pair or lone window): qkv, per-window V + pipelined
                heads + proj + scatter."""
                nwin = len(wins)
                wfree = N * nwin
                FPAD = 400 if nwin == 2 else 208   # fp8 Ko-step must be %16
                xwtb = pxwt.tile([128, 6, FPAD], F8, tag="xwtb")
                # qk2: slots 0:12 = per-head [q(64); Eq(32)], 12:24 = [k; Ek]
                qk2 = pqk.tile([128, 24, FPAD], F8, tag="qk2")
                att = pxwt.tile([128, 6, FPAD], F8, tag="att")

                # rel-pos bias operands into the qk2 bias rows 64:96
                # (one 3D DMA per side covers the whole group)
                gid = (wins[0] // NWS) * 3 + {0: 0, 2: 1, 4: 2}[wins[0] % NWS]
                nc.sync.dma_start(out=qk2[64:96, 0:12, 0:wfree],
                                  in_=eq_d[gid, :, :, 0:wfree].rearrange("h r i -> r h i"))
                nc.sync.dma_start(out=qk2[64:96, 12:24, 0:wfree],
                                  in_=ek_d[gid, :, :, 0:wfree].rearrange("h r i -> r h i"))

                # gather + transpose into xwtb (always composite: xn1 is padded)
                for ww_i, w in enumerate(wins):
                    woff = ww_i * N
                    wr, wc = _win_rc(w)
                    xw = pxw.tile([128, 2, DIM], F8, tag="xw")
                    for c in range(2):
                        nc.gpsimd.dma_start(
                            out=xw[0:CH, c, :],
                            in_=xn1_img[wr * WS + c * 7:wr * WS + c * 7 + 7,
                                        wc * WS:wc * WS + WS, :])
                    # transpose via regular identity matmul: out = xw_slice.T @ I
                    for c, coff in ((0, 0), (1, CH)):
                        for j in range(6):
                            pt = ptb.tile([128, 128], F32, tag="pt")
                            nc.tensor.matmul(
                                pt[0:128, 0:CH],
                                lhsT=xw[0:CH, c, j * 128:(j + 1) * 128],
                                rhs=ident[0:CH, 0:CH],
                                start=True, stop=True)
                            nc.vector.tensor_copy(
                                out=xwtb[:, j, woff + coff:woff + coff + CH],
                                in_=pt[0:128, 0:CH])

                # qkv^T for the whole group (fp8 DoubleRow over k-tile pairs);
                # evacuation splits each 2-head PSUM block into per-head slots:
                # even half on ACT (aligned), odd half on DVE (partition-shift)
                for oc in range(12):
                    pqm = psb.tile([128, 392], F32, tag="ps")
                    for kp in range(3):
                        nc.tensor.matmul(
                            pqm[:, 0:wfree],
                            lhsT=wqk_sb[:, 2 * kp:2 * kp + 2, oc * 128:(oc + 1) * 128],
                            rhs=xwtb[:, 2 * kp:2 * kp + 2, 0:wfree],
                            perf_mode=DR,
                            start=(kp == 0), stop=(kp == 2))
                    slot = 2 * (oc % 6) + (12 if oc >= 6 else 0)
                    nc.scalar.activation(out=qk2[0:64, slot, 0:wfree],
                                         in_=pqm[0:64, 0:wfree],
                                         func=mybir.ActivationFunctionType.Identity,
                                         bias=bqk_sb[0:64, oc:oc + 1], scale=inv_w)
                    nc.vector.tensor_scalar(out=qk2[0:64, slot + 1, 0:wfree],
                                            in0=pqm[64:128, 0:wfree],
                                            scalar1=inv_w,
                                            scalar2=bqk_sb[64:128, oc:oc + 1],
                                            op0=mybir.AluOpType.mult,
                                            op1=mybir.AluOpType.add)

                for ww_i, w in enumerate(wins):
                    woff = ww_i * N
                    # V (fp8): all heads + 64 ones columns for the denominator
                    va = pv.tile([128, 2, DIM + 64], F8, tag="va")
                    for c, coff in ((0, 0), (1, CH)):
                        nc.gpsimd.memset(va[0:CH, c, DIM:DIM + 64], 1.0)
                        pv0 = psb.tile([128, 384], F32, tag="ps")
                        pv1 = psb.tile([128, 384], F32, tag="ps")
                        for kp in range(3):
                            nc.tensor.matmul(
                                pv0[0:CH, :],
                                lhsT=xwtb[:, 2 * kp:2 * kp + 2,
                                          woff + coff:woff + coff + CH],
                                rhs=wv_sb[:, 2 * kp:2 * kp + 2, 0:384],
                                perf_mode=DR,
                                start=(kp == 0), stop=(kp == 2))
                            nc.tensor.matmul(
                                pv1[0:CH, :],
                                lhsT=xwtb[:, 2 * kp:2 * kp + 2,
                                          woff + coff:woff + coff + CH],
                                rhs=wv_sb[:, 2 * kp:2 * kp + 2, 384:768],
                                perf_mode=DR,
                                start=(kp == 0), stop=(kp == 2))
                        for half, pvm in ((0, pv0), (1, pv1)):
                            if _CACHE.get('vb_zero'):
                                nc.vector.tensor_scalar(
                                    out=va[0:CH, c, half * 384:(half + 1) * 384],
                                    in0=pvm[0:CH, :], scalar1=inv_w, scalar2=None,
                                    op0=mybir.AluOpType.mult)
                            else:
                                nc.vector.scalar_tensor_tensor(
                                    out=va[0:CH, c, half * 384:(half + 1) * 384],
                                    in0=pvm[0:CH, :],
                                    scalar=inv_w,
                                    in1=vb_sb[0:CH, half * 384:(half + 1) * 384],
                                    op0=mybir.AluOpType.mult,
                                    op1=mybir.AluOpType.add)

                    # heads: merged QK+bias for pair p, then PV/normalize p-1
                    pTs = {}
                    psos = {}

                    def emit_qk(h):
                        pss = psb.tile([128, 2 * N], F32, tag="ps")
                        for c in range(2):
                            nc.tensor.matmul(
                                pss[0:CH, c * N:(c + 1) * N],
                                lhsT=qk2[0:96, 12 + h,
                                         woff + c * CH:woff + c * CH + CH],
                                rhs=qk2[0:96, h, woff:woff + N],
                                start=True, stop=True)
                        pT = phd.tile([128, 2, 208], F8, tag="pT")
                        nc.scalar.activation(out=pT[0:CH, :, 0:N], in_=pss[0:CH, 0:2 * N],
                                             func=mybir.ActivationFunctionType.Exp)
                        pTs[h] = pT

                    def emit_pv(p):
                        pso = psb.tile([128, 2 * N], F32, tag="ps")
                        for h in (2 * p, 2 * p + 1):
                            b0 = (h % 2) * 64
                            pT = pTs.pop(h)
                            if b0 == 0:
                                # DoubleRow folds both key-chunks into one pass
                                nc.tensor.matmul(pso[0:64, 0:N],
                                                 lhsT=va[0:CH, 0:2, h * 64:(h + 1) * 64],
                                                 rhs=pT[0:CH, 0:2, 0:N],
                                                 perf_mode=DR, start=True, stop=True,
                                                 skip_group_check=True)
                                nc.tensor.matmul(pso[0:64, N:2 * N],
                                                 lhsT=va[0:CH, 0:2, DIM:DIM + 64],
                                                 rhs=pT[0:CH, 0:2, 0:N],
                                                 perf_mode=DR, start=True, stop=True,
                                                 skip_group_check=True)
                            else:
                                # walrus rejects DoubleRow + col-offset
                                # tile_position; plain fp8 per chunk instead
                                for c in range(2):
                                    nc.tensor.matmul(pso[64:128, 0:N],
                                                     lhsT=va[0:CH, c, h * 64:(h + 1) * 64],
                                                     rhs=pT[0:CH, c, 0:N],
                                                     start=(c == 0), stop=(c == 1),
                                                     skip_group_check=True)
                                    nc.tensor.matmul(pso[64:128, N:2 * N],
                                                     lhsT=va[0:CH, c, DIM:DIM + 64],
                                                     rhs=pT[0:CH, c, 0:N],
                                                     start=(c == 0), stop=(c == 1),
                                                     skip_group_check=True)
                        psos[p] = pso

                    def emit_norm(p):
                        pso = psos.pop(p)
                        rb = phd.tile([128, N], F32, tag="rb")
                        nc.vector.reciprocal(out=rb[:], in_=pso[:, N:2 * N])
                        nc.vector.tensor_mul(out=att[:, p, woff:woff + N],
                                             in0=pso[:, 0:N], in1=rb[:])

                    for p in range(6):
                        emit_qk(2 * p)
                        emit_qk(2 * p + 1)
                        if p >= 1:
                            emit_pv(p - 1)
                            emit_norm(p - 1)
                    emit_pv(5)
                    emit_norm(5)

                    # proj (fp8 DoubleRow) -> ow, then unpartition scatter
                    ow = pow_.tile([128, 2, DIM], BF16, tag="ow")
                    for c, coff in ((0, 0), (1, CH)):
                        pp0 = psb.tile([128, 384], F32, tag="ps")
                        pp1 = psb.tile([128, 384], F32, tag="ps")
                        for kp in range(3):
                            nc.tensor.matmul(
                                pp0[0:CH, :],
                                lhsT=att[:, 2 * kp:2 * kp + 2,
                                         woff + coff:woff + coff + CH],
                                rhs=wp_sb[:, 2 * kp:2 * kp + 2, 0:384],
                                perf_mode=DR,
                                start=(kp == 0), stop=(kp == 2))
                            nc.tensor.matmul(
                                pp1[0:CH, :],
                                lhsT=att[:, 2 * kp:2 * kp + 2,
                                         woff + coff:woff + coff + CH],
                                rhs=wp_sb[:, 2 * kp:2 * kp + 2, 384:768],
                                perf_mode=DR,
                                start=(kp == 0), stop=(kp == 2))
                        for half, psp in ((0, pp0), (1, pp1)):
                            if _CACHE.get('pb_zero'):
                                nc.scalar.activation(
                                    out=ow[0:CH, c, half * 384:(half + 1) * 384],
                                    in_=psp[0:CH, :],
                                    func=mybir.ActivationFunctionType.Copy,
                                    bias=0.0, scale=inv_w)
                            else:
                                nc.vector.scalar_tensor_tensor(
                                    out=ow[0:CH, c, half * 384:(half + 1) * 384],
                                    in0=psp[0:CH, :], scalar=inv_w,
                                    in1=pb_sb[0:CH, half * 384:(half + 1) * 384],
                                    op0=mybir.AluOpType.mult,
                                    op1=mybir.AluOpType.add)
                    wr, wc = _win_rc(w)
                    for c in range(2):
                        nc.gpsimd.dma_start(
                            out=at_img[wr * WS + c * 7:wr * WS + c * 7 + 7,
                                       wc * WS:wc * WS + WS, :],
                            in_=ow[0:CH, c, :])

            def emit_c_group(g):
                """Phase C for token tiles 4g..4g+3 (512 tokens = 8 image
                rows): residual, LN2, MLP, out.  4-tile grouping halves the
                gelu / bn / DMA fixed costs vs per-2-tile groups."""
                xg = pg.tile([128, 4, DIM], F32, tag="xg")
                ag = pag.tile([128, 4, DIM], BF16, tag="ag")
                nc.gpsimd.dma_start(out=xg[:], in_=x_pt[:, 4 * g:4 * g + 4, :])
                for a in range(4):
                    r0 = 8 * g + 2 * a
                    nc.gpsimd.dma_start(out=ag[:, a, :],
                                        in_=at_img[r0:r0 + 2, 0:HH, :])
                # x2 = x + attn (in place into xg)
                nc.vector.tensor_add(out=xg[:, :, :], in0=xg[:, :, :], in1=ag[:, :, :])
                xn2t = pgx.tile([128, 6, 512], F8, tag="xn2t")
                mvc = pg.tile([128, 2, 4], F32, tag="mvc")
                for s in range(4):
                    st = pg.tile([128, 2, 6], F32, tag="stC")
                    for sub in range(2):
                        nc.vector.bn_stats(out=st[:, sub, :],
                                           in_=xg[:, s, sub * 384:(sub + 1) * 384])
                    nc.vector.bn_aggr(out=mvc[:, :, s], in_=st[:])
                sdc = pg.tile([128, 4], F32, tag="sdC")
                nc.scalar.activation(out=sdc[:], in_=mvc[:, 1, :],
                                     func=mybir.ActivationFunctionType.Sqrt,
                                     bias=eps_t[:], scale=1.0)
                rsc = pg.tile([128, 4], F32, tag="rsC")
                nc.vector.reciprocal(out=rsc[:], in_=sdc[:])
                for s in range(4):
                    xn2b = pg.tile([128, DIM], F8, tag="xn2b")
                    nc.vector.tensor_scalar(out=xn2b[:, :], in0=xg[:, s, :],
                                            scalar1=mvc[:, 0, s:s + 1],
                                            scalar2=rsc[:, s:s + 1],
                                            op0=mybir.AluOpType.subtract,
                                            op1=mybir.AluOpType.mult)
                    if not _CACHE.get('b2_zero'):
                        nc.vector.tensor_add(out=xg[:, s, :], in0=xg[:, s, :],
                                             in1=b2_sb[:])
                    for j in range(6):
                        pt = ptb.tile([128, 128], F32, tag="pt")
                        nc.tensor.matmul(pt[:, :],
                                         lhsT=xn2b[:, j * 128:(j + 1) * 128],
                                         rhs=ident[:, :], start=True, stop=True)
                        nc.vector.tensor_copy(out=xn2t[:, j, s * 128:(s + 1) * 128],
                                              in_=pt[:, :])
                h1t = ph.tile([128, 24, 512], BF16, tag="h1t")
                for oc in range(24):
                    psh = psb.tile([128, 512], F32, tag="ps")
                    for kp in range(3):
                        nc.tensor.matmul(
                            psh[:, :],
                            lhsT=w1_sb[:, 2 * kp:2 * kp + 2, oc * 128:(oc + 1) * 128],
                            rhs=xn2t[:, 2 * kp:2 * kp + 2, :],
                            perf_mode=DR,
                            start=(kp == 0), stop=(kp == 2))
                    if sig_gelu:
                        # CoreSim lacks Gelu; x*sigmoid(1.702x) validates shapes
                        hpre = pg.tile([128, 512], BF16, tag="hpre")
                        nc.scalar.activation(out=hpre[:], in_=psh[:, :],
                                             func=mybir.ActivationFunctionType.Identity,
                                             bias=b1_sb[:, oc:oc + 1], scale=inv_w)
                        sg = pg.tile([128, 512], BF16, tag="sg")
                        nc.scalar.activation(out=sg[:], in_=hpre[:],
                                             func=mybir.ActivationFunctionType.Sigmoid,
                                             bias=0.0, scale=1.702)
                        nc.vector.tensor_mul(out=h1t[:, oc, :], in0=hpre[:], in1=sg[:])
                    else:
                        nc.scalar.activation(out=h1t[:, oc, :], in_=psh[:, :],
                                             func=mybir.ActivationFunctionType.Gelu,
                                             bias=b1_sb[:, oc:oc + 1], scale=inv_w)
                for sp in range(2):
                    yo = py.tile([128, 2, DIM], F32, tag="yo")
                    for ss in range(2):
                        s = 2 * sp + ss
                        pf0 = psb.tile([128, 384], F32, tag="ps")
                        pf1 = psb.tile([128, 384], F32, tag="ps")
                        for kt in range(24):
                            nc.tensor.matmul(
                                pf0[:, :],
                                lhsT=h1t[:, kt, s * 128:(s + 1) * 128],
                                rhs=w2_sb[:, kt, 0:384],
                                start=(kt == 0), stop=(kt == 23))
                            nc.tensor.matmul(
                                pf1[:, :],
                                lhsT=h1t[:, kt, s * 128:(s + 1) * 128],
                                rhs=w2_sb[:, kt, 384:768],
                                start=(kt == 0), stop=(kt == 23))
                        for half, psf in ((0, pf0), (1, pf1)):
                            nc.vector.tensor_add(
                                out=yo[:, ss, half * 384:(half + 1) * 384],
                                in0=psf[:, :],
                                in1=xg[:, s, half * 384:(half + 1) * 384])
                    nc.gpsimd.dma_start(out=y_pt[:, 4 * g + 2 * sp:4 * g + 2 * sp + 2, :],
                                        in_=yo[:])

            # phase C double-group G covers image rows 8G..8G+8; ready once
            # the band containing its last row is done.  Groups are POPPED
            # one band later (after that band's last window group) so band
            # boundaries always have PE-ready work.
            c_ready = {0: [0], 1: [1, 2], 2: [3, 4], 3: [5, 6], 4: [7]}
            pending = []

            emit_ln1_band(0)
            for band in range(5):
                w0 = band * NWS
                emit_window_group((w0, w0 + 1))
                if band == 0:
                    emit_bulk_weights()
                # overlap next band's LN1 (DVE/DMA) with this band's windows
                if band < 4:
                    emit_ln1_band(band + 1)
                emit_window_group((w0 + 2, w0 + 3))
                emit_window_group((w0 + 4,))
                while pending:
                    emit_c_group(pending.pop(0))
                pending.extend(c_ready[band])
            for g in pending:
                emit_c_group(g)

    if os.environ.get('KERNEL_NOLDDEDUP') != '1':
        _dedup_ldweights(nc)
    if os.environ.get('KERNEL_SIM') != '1':
        _split_waits(nc)
    _CACHE['nc'] = nc
    return nc


def _host_prep(inputs):
    """Fold LN affines into matmul weights, build rel-pos operands."""
    f32 = np.float32
    x = np.asarray(inputs['x'], f32)
    q_idx = np.asarray(inputs['q_idx']).astype(np.int64)
    k_idx = np.asarray(inputs['k_idx']).astype(np.int64)
    ln1_w = np.asarray(inputs['ln1_w'], f32); ln1_b = np.asarray(inputs['ln1_b'], f32)
    ln2_w = np.asarray(inputs['ln2_w'], f32); ln2_b = np.asarray(inputs['ln2_b'], f32)
    qkv_w = np.asarray(inputs['qkv_w'], f32); qkv_b = np.asarray(inputs['qkv_b'], f32)
    proj_w = np.asarray(inputs['proj_w'], f32); proj_b = np.asarray(inputs['proj_b'], f32)
    mlp_w1 = np.asarray(inputs['mlp_w1'], f32); mlp_b1 = np.asarray(inputs['mlp_b1'], f32)
    mlp_w2 = np.asarray(inputs['mlp_w2'], f32); mlp_b2 = np.asarray(inputs['mlp_b2'], f32)
    rel_h = np.asarray(inputs['rel_h'], f32); rel_w = np.asarray(inputs['rel_w'], f32)

    scale = HD ** -0.5
    Wqkv = ln1_w[:, None] * qkv_w
    bqkv = ln1_b @ qkv_w + qkv_b
    Wqkv = Wqkv.copy(); bqkv = bqkv.copy()
    Wqkv[:, :DIM] *= scale
    bqkv[:DIM] *= scale
    W1 = ln2_w[:, None] * mlp_w1
    b1 = ln2_b @ mlp_w1 + mlp_b1

    coords = np.arange(WS)[:, None] - np.arange(WS)[None, :] + (WS - 1)
    Sh = rel_h[coords].sum(-1).astype(f32)
    Sw = rel_w[coords].sum(-1).astype(f32)

    qr, qc = q_idx // WS, q_idx % WS
    kr, kc = k_idx // WS, k_idx % WS
    nb = q_idx.shape[0]
    Eq = np.zeros((nb, EB, N), f32)
    Eq[:, 0:WS, :] = np.take(Sh, qr, axis=0).transpose(0, 2, 1)
    Eq[:, WS:2 * WS, :] = np.take(Sw, qc, axis=0).transpose(0, 2, 1)
    Ek = np.zeros((nb, EB, N), f32)
    bi = np.arange(nb)[:, None]
    ar = np.arange(N)[None, :]
    Ek[bi, kr, ar] = 1.0
    Ek[bi, WS + kc, ar] = 1.0

    bf = ml_dtypes.bfloat16
    f8 = ml_dtypes.float8_e4m3fn
    shared = {
        "wqk": np.ascontiguousarray(Wqkv[:, :2 * DIM] * WSCALE).astype(f8),
        "wv": np.ascontiguousarray(Wqkv[:, 2 * DIM:] * WSCALE).astype(f8),
        "bqk": np.ascontiguousarray(bqkv[:2 * DIM].reshape(12, 128)),
        "vb": np.ascontiguousarray(bqkv[2 * DIM:].reshape(1, DIM)),
        "wp": np.ascontiguousarray(proj_w * WSCALE).astype(f8),
        "pb": proj_b.reshape(1, DIM).copy(),
        "w1": np.ascontiguousarray(W1 * WSCALE).astype(f8),
        "b1": np.ascontiguousarray(b1.reshape(24, 128)),
        "w2": mlp_w2.astype(bf),
        "b2": mlp_b2.reshape(1, DIM).copy(),
    }
    Eq = Eq.astype(f8).reshape(B, NW, NH, EB, N)
    Ek = Ek.astype(f8).reshape(B, NW, NH, EB, N)
    # regroup per window-group (pair, pair, single per band), windows'
    # key columns contiguous on the last axis
    EqG = np.zeros((B, 15, NH, EB, 2 * N), f8)
    EkG = np.zeros((B, 15, NH, EB, 2 * N), f8)
    for band in range(5):
        for gi, ws_ in enumerate(((0, 1), (2, 3), (4,))):
            g = band * 3 + gi
            for wi, wo in enumerate(ws_):
                w = band * NWS + wo
                EqG[:, g, :, :, wi * N:(wi + 1) * N] = Eq[:, w]
                EkG[:, g, :, :, wi * N:(wi + 1) * N] = Ek[:, w]
    in_maps = []
    for b in range(B):
        m = dict(shared)
        m["x"] = np.ascontiguousarray(x[b].reshape(NTOK, DIM))
        m["xbf"] = np.ascontiguousarray(x[b].reshape(NTOK, DIM)).astype(bf)
        m["eq"] = np.ascontiguousarray(EqG[b])
        m["ek"] = np.ascontiguousarray(EkG[b])
        in_maps.append(m)
    return in_maps


def kernel(**inputs):
    in_maps = _host_prep(inputs)
    if 'nc' not in _CACHE:
        _CACHE['pb_zero'] = not np.any(np.asarray(in_maps[0]['pb'], np.float32))
        _CACHE['b2_zero'] = not np.any(np.asarray(in_maps[0]['b2'], np.float32))
    nc = _build()
    trace = os.environ.get('KERNEL_TRACE') == '1'
    if trace:
        _install_ntff_hook()
    res = run_bass_kernel_spmd(nc, in_maps, list(range(B)), trace=trace)
    if trace and res.exec_time_ns is not None:
        print(f"HW exec time: {res.exec_time_ns} ns")
        _CACHE['exec_time_ns'] = res.exec_time_ns
    _CACHE['last_results'] = res
    out = np.stack([np.asarray(res.results[b]["y"]).reshape(HH, WW, DIM)
                    for b in range(B)])
    return out.astype(np.float32)


# revision 37
# speedup vs baseline: 1.0258x; 1.0020x over previous
"""Trainium2 Bass kernel for nn_Block_72138270704025 (windowed sparse attention
block: LN1 -> window partition -> MHA with decomposed rel-pos bias gathered by
q_idx/k_idx -> window unpartition -> residual -> LN2 -> MLP(gelu) -> residual).

Sharding: data-parallel over batch B=8, one batch element per NeuronCore; all
weights replicated.  Host folds LN affine params into the adjacent matmul
weights, precomputes the rel-pos tables Sh/Sw, and turns the per-(window,head)
index gathers into small per-window fp8 operands so the bias folds into the
logits matmul.

v3 changes over v2:
- Merged QK+bias logits: per head the contraction is the 96-row concat
  [k_h(64); Ek_h(28+4 zero pad)] x [q_h(64); Eq_h(32)], so ONE matmul per
  (head, key-chunk) produces logits+bias (v2 used two).  Halves the logits
  matmul and ldweights count.  Odd heads' q/k are evacuated from PSUM rows
  64:128 to SBUF rows 0:64 with a partition-shifted DVE op (64-channel DVE
  ops may read any source partition window); even heads evacuate on ACT.
- Phase C (residual+LN2+MLP) groups deferred one band and interleaved
  BETWEEN window groups, so band boundaries always have ready PE work and
  the serial scatter->load->LN2 chain of a fresh c-group is hidden.
- xn1 and attn intermediates stored as zero-padded 70x70 images in DRAM:
  every window gather/scatter is 2 composite 3D DMAs (v2 used per-row DMAs
  plus memsets for the 9 edge windows), shortening the GpSimd DMA queue.

Carried over from v2: fp8e4m3 DoubleRow matmuls for qkv/v/proj/fc1 and PV
(weights x32 host scale), softmax reciprocal per head-pair, head software
pipelining, transposes as identity matmuls, batched LN sqrt, bf16 fc2.
"""
import os
import sys

for _p in ('/opt/trn_rl_repo', '/root/.axon_site/_ro/trn_rl_repo'):
    if os.path.isdir(_p) and _p not in sys.path:
        sys.path.append(_p)

import numpy as np
import ml_dtypes

import concourse.bass as bass
import concourse.tile as tile
from concourse import mybir
from concourse.bass_utils import run_bass_kernel_spmd
from concourse.tile import ScopedClock
from concourse.masks import make_identity

# ---- problem constants (hardcoded per contest rules) ----
B = 8
HH = 64
WW = 64
DIM = 768
NH = 12
WS = 14
HD = 64
N = 196            # tokens per window
NWS = 5            # window grid side
NW = 25            # windows per image
EPS = 1e-5
NTOK = HH * WW     # 4096 tokens per core
CH = 98            # window token chunk: 7 rows of 14 (196 = 2x98)
WSCALE = 32.0      # host fp8 weight scale (descaled at PSUM evacuation)
HP = 70            # padded image side (5 windows x 14)
EB = 32            # bias contraction rows (28 used + 4 zero pad)

F32 = mybir.dt.float32
BF16 = mybir.dt.bfloat16
F8 = mybir.dt.float8e4
DR = mybir.MatmulPerfMode.DoubleRow


def _patch_tile_drain():
    """Walrus CoreV3 codegen rejects a Drain carrying multiple sem waits
    ("Too many sync wait commands").  Emit explicit wait_ge instructions
    before the kernel-tail drain instead."""
    if getattr(tile.TileContext, '_drain_patched', False):
        return

    def _drain_and_barrier(self, tick_clock, wait_clock):
        nc = self.nc
        dummy = nc.sync.nop(nofuse=True)
        wait_clock.add_sem_waits(dummy.ins, ScopedClock({None: tick_clock.global_clock}))
        waits = list(dummy.ins.sync_info.on_wait or [])
        dummy.ins.sync_info.on_wait = []
        assert self.sems is not None
        by_id = {}
        for h in self.sems.allocated().values():
            by_id[getattr(h, 'id', None)] = h
            by_id[getattr(h, 'name', None)] = h
        for w in waits:
            h = by_id.get(w.id) or by_id.get(w.ant_name)
            assert h is not None, (w.id, w.ant_name)
            nc.sync.wait_ge(h, w.wait_value)
        nc.sync.drain()
        nc.all_engine_barrier()
        popped = nc._tile_sem_poison_stack.pop()
        assert popped is self._sem_poison
        nc.clear_and_free_semaphores(list(self.sems.allocated().values()))
        nc.all_engine_barrier()

    tile.TileContext._drain_and_barrier = _drain_and_barrier
    tile.TileContext._drain_patched = True


def _install_ntff_hook():
    """Recreate the missing antenv.axon_hooks module so trace=True can reach
    the axon NTFF profiler (used only when KERNEL_TRACE=1)."""
    try:
        import types
        import antenv
        if 'antenv.axon_hooks' in sys.modules:
            return True
        mod = types.ModuleType('antenv.axon_hooks')
        mod._hook = None
        mod.set_axon_ntff_profile_hook = lambda h: setattr(mod, '_hook', h)
        mod.get_axon_ntff_profile_hook = lambda: mod._hook
        sys.modules['antenv.axon_hooks'] = mod
        antenv.axon_hooks = mod
        from trn_agent_boot.trn_boot import _ntff_profile_via_ctypes
        mod._hook = _ntff_profile_via_ctypes('/opt/axon/libaxon_pjrt.so')
        return mod._hook is not None
    except Exception:
        return False


# window geometry helpers
def _win_rc(w):
    return w // NWS, w % NWS


_CACHE = {}


def _dedup_ldweights(nc):
    """Tile lowers each matmul to Ldweights+Matmult.  Back-to-back matmuls
    that share a stationary operand reload identical weights; drop the
    redundant Ldweights (keeping its sem waits / updates on a zero-cost
    EventSemaphore)."""
    ndrop = 0
    for fn in nc.m.functions:
        for blk in fn.blocks:
            insts = blk.instructions
            out = []
            prev_key = None
            dirty = False
            for ins in insts:
                if ins.engine != mybir.EngineType.PE:
                    out.append(ins)
                    continue
                if ins.opcode == 'Ldweights':
                    a = ins.ins[0]
                    key = (str(getattr(a, 'memory_location', None)),
                           getattr(a, 'offset', None), str(getattr(a, 'ap', None)),
                           str(getattr(ins, 'is_transpose', None)),
                           str(getattr(ins, 'perf_mode', None)))
                    si = ins.sync_info
                    has_sync = si and (si.on_wait or si.on_update)
                    if key == prev_key:
                        ndrop += 1
                        dirty = True
                        if has_sync:
                            ev = mybir.InstEventSemaphore(
                                name=f"LDDROP-{nc.next_id()}", ins=[], outs=[])
                            ev.engine = ins.engine
                            ev.sync_info = mybir.SyncInfo(
                                on_wait=list(si.on_wait or []),
                                on_update=list(si.on_update or []))
                            out.append(ev)
                        continue
                    prev_key = key
                    out.append(ins)
                elif ins.opcode == 'Matmult' and not getattr(ins, 'is_transpose', False):
                    out.append(ins)
                else:
                    prev_key = None
                    out.append(ins)
            if dirty:
                blk.instructions = out
    return ndrop


def _split_waits(nc, cap=None):
    """Walrus CoreV2/V3 codegen rejects instructions whose sync_info carries
    more waits than the per-opcode ISA ctrl struct holds.  Hoist excess waits
    onto standalone EventSemaphore instructions."""
    if cap is None:
        cap = int(os.environ.get('KERNEL_MAXWAITS', '1'))
    n_split = 0
    for fn in nc.m.functions:
        for blk in fn.blocks:
            insts = blk.instructions
            out = []
            dirty = False
            for ins in insts:
                si = ins.sync_info
                waits = list(si.on_wait) if si and si.on_wait else []
                # InstISA (custom DVE ops) cannot encode sem waits at all
                limit = 0 if ins.opcode == 'ISA' else (
                    1 if ins.opcode in ('Drain',) else cap)
                if len(waits) > limit:
                    keep, extra = waits[:limit], waits[limit:]
                    for k in range(0, len(extra), cap):
                        ev = mybir.InstEventSemaphore(
                            name=f"WSPLIT-{nc.next_id()}", ins=[], outs=[])
                        ev.engine = ins.engine
                        ev.sync_info = mybir.SyncInfo(
                            on_wait=extra[k:k + cap], on_update=[])
                        out.append(ev)
                        n_split += 1
                    si.on_wait = keep
                    dirty = True
                out.append(ins)
            if dirty:
                blk.instructions = out
    return n_split


def _build():
    if 'nc' in _CACHE:
        return _CACHE['nc']
    _patch_tile_drain()

    nc = bass.Bass()

    # ---- dram parameters ----
    x_d = nc.dram_tensor("x", [NTOK, DIM], F32, kind="ExternalInput")
    xbf_d = nc.dram_tensor("xbf", [NTOK, DIM], BF16, kind="ExternalInput")
    # per window-GROUP (3 per band: pair, pair, single) with the windows'
    # key columns pre-interleaved so one 3D DMA loads the whole group
    eq_d = nc.dram_tensor("eq", [15, NH, EB, 2 * N], F8, kind="ExternalInput")
    ek_d = nc.dram_tensor("ek", [15, NH, EB, 2 * N], F8, kind="ExternalInput")
    wqk_d = nc.dram_tensor("wqk", [DIM, 2 * DIM], F8, kind="ExternalInput")
    wv_d = nc.dram_tensor("wv", [DIM, DIM], F8, kind="ExternalInput")
    bqk_d = nc.dram_tensor("bqk", [128, 12], F32, kind="ExternalInput")
    vb_d = nc.dram_tensor("vb", [1, DIM], F32, kind="ExternalInput")
    wp_d = nc.dram_tensor("wp", [DIM, DIM], F8, kind="ExternalInput")
    pb_d = nc.dram_tensor("pb", [1, DIM], F32, kind="ExternalInput")
    w1_d = nc.dram_tensor("w1", [DIM, 4 * DIM], F8, kind="ExternalInput")
    b1_d = nc.dram_tensor("b1", [128, 24], F32, kind="ExternalInput")
    w2_d = nc.dram_tensor("w2", [4 * DIM, DIM], BF16, kind="ExternalInput")
    b2_d = nc.dram_tensor("b2", [1, DIM], F32, kind="ExternalInput")
    y_d = nc.dram_tensor("y", [NTOK, DIM], F32, kind="ExternalOutput")

    dbg = os.environ.get('KERNEL_DEBUG') == '1'
    skind = dict(kind="ExternalOutput") if dbg else {}
    # padded 70x70 images for the LN1 output and the attention output:
    # pad region of xn1 is zeroed once so every gather/scatter is composite
    xn1_d = nc.dram_tensor("xn1", [HP * HP, DIM], F8)
    at_d = nc.dram_tensor("attn", [HP * HP, DIM], BF16, **skind)

    xbf_t32 = xbf_d.rearrange("(a p) d -> a p d", p=128)  # 32 token tiles
    x_pt = x_d.rearrange("(a p) d -> p a d", p=128)       # grouped loads
    xn1_img = xn1_d.rearrange("(r c) d -> r c d", c=HP)
    at_img = at_d.rearrange("(r c) d -> r c d", c=HP)
    y_pt = y_d.rearrange("(a p) d -> p a d", p=128)

    inv_w = 1.0 / WSCALE

    with tile.TileContext(nc, pool_alloc_mode='queue') as tc:
        with tc.tile_pool(name="cW", bufs=1) as pcw, \
             tc.tile_pool(name="lnA", bufs=4) as pa, \
             tc.tile_pool(name="xtP", bufs=7) as pxt, \
             tc.tile_pool(name="xwP", bufs=2) as pxw, \
             tc.tile_pool(name="xwtP", bufs=2) as pxwt, \
             tc.tile_pool(name="qkP", bufs=2) as pqk, \
             tc.tile_pool(name="vP", bufs=2) as pv, \
             tc.tile_pool(name="hdP", bufs=4) as phd, \
             tc.tile_pool(name="owP", bufs=2) as pow_, \
             tc.tile_pool(name="gC", bufs=2) as pg, \
             tc.tile_pool(name="agC", bufs=1) as pag, \
             tc.tile_pool(name="yC", bufs=1) as py, \
             tc.tile_pool(name="hC", bufs=1) as ph, \
             tc.tile_pool(name="gX", bufs=1) as pgx, \
             tc.tile_pool(name="psB", bufs=6, space="PSUM") as psb, \
             tc.tile_pool(name="ptB", bufs=2, space="PSUM") as ptb:

            # ---- persistent weights / consts.  Attention weights go early
            # on the sync HWDGE ring; the bulk MLP weights (only needed from
            # the first phase-C group, ~200us in) load on the gpsimd ring
            # AFTER the first window group's gathers (emit_bulk_weights). ----
            w1_sb = pcw.tile([128, 6, 4 * DIM], F8)
            b1_sb = pcw.tile([128, 24], F32)
            w2_sb = pcw.tile([128, 24, DIM], BF16)
            if not _CACHE.get('b2_zero'):
                b2_sb = pcw.tile([128, DIM], F32)

            def emit_bulk_weights():
                nc.gpsimd.dma_start(out=w1_sb[:],
                                    in_=w1_d.rearrange("(k p) n -> p k n", p=128))
                nc.gpsimd.dma_start(out=b1_sb[:], in_=b1_d[:, :])
                nc.gpsimd.dma_start(out=w2_sb[:],
                                    in_=w2_d.rearrange("(k p) n -> p k n", p=128))
                if not _CACHE.get('b2_zero'):
                    nc.gpsimd.dma_start(out=b2_sb[:],
                                        in_=b2_d[0:1, :].to_broadcast((128, DIM)))

            eps_t = pcw.tile([128, 1], F32)
            nc.vector.memset(eps_t[:], EPS)
            ident = pcw.tile([128, 128], F8)
            make_identity(nc, ident[:])
            wqk_sb = pcw.tile([128, 6, 2 * DIM], F8)
            wv_sb = pcw.tile([128, 6, DIM], F8)
            wp_sb = pcw.tile([128, 6, DIM], F8)
            bqk_sb = pcw.tile([128, 12], F32)
            zt = pcw.tile([128, DIM], F8)
            nc.vector.memset(zt[:], 0.0)
            if not _CACHE.get('vb_zero'):
                vb_sb = pcw.tile([128, DIM], F32)
            if not _CACHE.get('pb_zero'):
                pb_sb = pcw.tile([128, DIM], F32)

            def emit_attn_weights():
                """Sync-ring weight loads, emitted AFTER band-0's x loads so
                LN1 starts immediately at kernel start."""
                nc.sync.dma_start(out=wqk_sb[:],
                                  in_=wqk_d.rearrange("(k p) n -> p k n", p=128))
                nc.sync.dma_start(out=wv_sb[:],
                                  in_=wv_d.rearrange("(k p) n -> p k n", p=128))
                nc.sync.dma_start(out=wp_sb[:],
                                  in_=wp_d.rearrange("(k p) n -> p k n", p=128))
                nc.sync.dma_start(out=bqk_sb[:], in_=bqk_d[:, :])
                if not _CACHE.get('vb_zero'):
                    nc.gpsimd.dma_start(out=vb_sb[:],
                                        in_=vb_d[0:1, :].to_broadcast((128, DIM)))
                if not _CACHE.get('pb_zero'):
                    nc.gpsimd.dma_start(out=pb_sb[:],
                                        in_=pb_d[0:1, :].to_broadcast((128, DIM)))

            def emit_zero_pad():
                """Zero the xn1 pad region once (right pad cols 64:70 of rows
                0:64, bottom rows 64:70) so edge-window gathers read exact
                zeros.  Emitted after wg(0,1) — first needed by wg(4)."""
                for i in range(4):
                    nc.gpsimd.dma_start(out=xn1_img[16 * i:16 * i + 16, HH:HP, :],
                                        in_=zt[0:96, :])
                for r in range(HH, HP):
                    nc.gpsimd.dma_start(out=xn1_img[r:r + 1, 0:HP, :], in_=zt[0:HP, :])

            sig_gelu = os.environ.get('KERNEL_GELU') == 'sig'

            def emit_ln1_band(band):
                """LN1 for this band's token tiles; batched sqrt for the band."""
                band_tiles = [7, 7, 7, 7, 4]
                nbt = band_tiles[band]
                xts = []
                mvb = pa.tile([128, 2, 7], F32, tag="mvb")
                for bt in range(nbt):
                    t = band * 7 + bt
                    xt = pxt.tile([128, DIM], BF16, tag="xt")
                    nc.sync.dma_start(out=xt[:], in_=xbf_t32[t])
                    st = pa.tile([128, 2, 6], F32, tag="st")
                    for s in range(2):
                        nc.vector.bn_stats(out=st[:, s, :], in_=xt[:, s * 384:(s + 1) * 384])
                    nc.vector.bn_aggr(out=mvb[:, :, bt], in_=st[:])
                    xts.append(xt)
                sdb = pa.tile([128, 7], F32, tag="sdb")
                nc.scalar.activation(out=sdb[:, 0:nbt], in_=mvb[:, 1, 0:nbt],
                                     func=mybir.ActivationFunctionType.Sqrt,
                                     bias=eps_t[:], scale=1.0)
                rsd = pa.tile([128, 7], F32, tag="rsd")
                nc.vector.reciprocal(out=rsd[:, 0:nbt], in_=sdb[:, 0:nbt])
                for bt in range(nbt):
                    xn = pa.tile([128, DIM], F8, tag="xn")
                    nc.vector.tensor_scalar(out=xn[:], in0=xts[bt][:],
                                            scalar1=mvb[:, 0, bt:bt + 1],
                                            scalar2=rsd[:, bt:bt + 1],
                                            op0=mybir.AluOpType.subtract,
                                            op1=mybir.AluOpType.mult)
                    r0 = band * WS + 2 * bt
                    nc.sync.dma_start(out=xn1_img[r0:r0 + 2, 0:HH, :], in_=xn[:])

            def emit_window_group(wins):
                """One group (pair or lone window): qkv, per-window V + pipelined
                heads + proj + scatter."""
                nwin = len(wins)
                wfree = N * nwin
                FPAD = 400 if nwin == 2 else 208   # fp8 Ko-step must be %16
                xwtb = pxwt.tile([128, 6, FPAD], F8, tag="xwtb")
                # qk2: slots 0:12 = per-head [q(64); Eq(32)], 12:24 = [k; Ek]
                qk2 = pqk.tile([128, 24, FPAD], F8, tag="qk2")
                att = pxwt.tile([128, 6, FPAD], F8, tag="att")

                # rel-pos bias operands into the qk2 bias rows 64:96
                # (one 3D DMA per side covers the whole group)
                gid = (wins[0] // NWS) * 3 + {0: 0, 2: 1, 4: 2}[wins[0] % NWS]
                nc.sync.dma_start(out=qk2[64:96, 0:12, 0:wfree],
                                  in_=eq_d[gid, :, :, 0:wfree].rearrange("h r i -> r h i"))
                nc.sync.dma_start(out=qk2[64:96, 12:24, 0:wfree],
                                  in_=ek_d[gid, :, :, 0:wfree].rearrange("h r i -> r h i"))

                # gather + transpose into xwtb (always composite: xn1 is padded)
                for ww_i, w in enumerate(wins):
                    woff = ww_i * N
                    wr, wc = _win_rc(w)
                    xw = pxw.tile([128, 2, DIM], F8, tag="xw")
                    for c in range(2):
                        nc.gpsimd.dma_start(
                            out=xw[0:CH, c, :],
                            in_=xn1_img[wr * WS + c * 7:wr * WS + c * 7 + 7,
                                        wc * WS:wc * WS + WS, :])
                    # transpose via regular identity matmul: out = xw_slice.T @ I
                    for c, coff in ((0, 0), (1, CH)):
                        for j in range(6):
                            pt = ptb.tile([128, 128], F32, tag="pt")
                            nc.tensor.matmul(
                                pt[0:128, 0:CH],
                                lhsT=xw[0:CH, c, j * 128:(j + 1) * 128],
                                rhs=ident[0:CH, 0:CH],
                                start=True, stop=True)
                            nc.vector.tensor_copy(
                                out=xwtb[:, j, woff + coff:woff + coff + CH],
                                in_=pt[0:128, 0:CH])

                # qkv^T for the whole group (fp8 DoubleRow over k-tile pairs);
                # evacuation splits each 2-head PSUM block into per-head slots:
                # even half on ACT (aligned), odd half on DVE (partition-shift)
                for oc in range(12):
                    pqm = psb.tile([128, 392], F32, tag="ps")
                    for kp in range(3):
                        nc.tensor.matmul(
                            pqm[:, 0:wfree],
                            lhsT=wqk_sb[:, 2 * kp:2 * kp + 2, oc * 128:(oc + 1) * 128],
                            rhs=xwtb[:, 2 * kp:2 * kp + 2, 0:wfree],
                            perf_mode=DR,
                            start=(kp == 0), stop=(kp == 2))
                    slot = 2 * (oc % 6) + (12 if oc >= 6 else 0)
                    nc.scalar.activation(out=qk2[0:64, slot, 0:wfree],
                                         in_=pqm[0:64, 0:wfree],
                                         func=mybir.ActivationFunctionType.Identity,
                                         bias=bqk_sb[0:64, oc:oc + 1], scale=inv_w)
                    nc.vector.tensor_scalar(out=qk2[0:64, slot + 1, 0:wfree],
                                            in0=pqm[64:128, 0:wfree],
                                            scalar1=inv_w,
                                            scalar2=bqk_sb[64:128, oc:oc + 1],
                                            op0=mybir.AluOpType.mult,
                                            op1=mybir.AluOpType.add)

                for ww_i, w in enumerate(wins):
                    woff = ww_i * N
                    # V (fp8): all heads + 64 ones columns for the denominator
                    va = pv.tile([128, 2, DIM + 64], F8, tag="va")
                    for c, coff in ((0, 0), (1, CH)):
                        nc.gpsimd.memset(va[0:CH, c, DIM:DIM + 64], 1.0)
                        pv0 = psb.tile([128, 384], F32, tag="ps")
                        pv1 = psb.tile([128, 384], F32, tag="ps")
                        for kp in range(3):
                            nc.tensor.matmul(
                                pv0[0:CH, :],
                                lhsT=xwtb[:, 2 * kp:2 * kp + 2,
                                          woff + coff:woff + coff + CH],
                                rhs=wv_sb[:, 2 * kp:2 * kp + 2, 0:384],
                                perf_mode=DR,
                                start=(kp == 0), stop=(kp == 2))
                            nc.tensor.matmul(
                                pv1[0:CH, :],
                                lhsT=xwtb[:, 2 * kp:2 * kp + 2,
                                          woff + coff:woff + coff + CH],
                                rhs=wv_sb[:, 2 * kp:2 * kp + 2, 384:768],
                                perf_mode=DR,
                                start=(kp == 0), stop=(kp == 2))
                        for half, pvm in ((0, pv0), (1, pv1)):
                            if _CACHE.get('vb_zero'):
                                nc.vector.tensor_scalar(
                                    out=va[0:CH, c, half * 384:(half + 1) * 384],
                                    in0=pvm[0:CH, :], scalar1=inv_w, scalar2=None,
                                    op0=mybir.AluOpType.mult)
                            else:
                                nc.vector.scalar_tensor_tensor(
                                    out=va[0:CH, c, half * 384:(half + 1) * 384],
                                    in0=pvm[0:CH, :],
                                    scalar=inv_w,
                                    in1=vb_sb[0:CH, half * 384:(half + 1) * 384],
                                    op0=mybir.AluOpType.mult,
                                    op1=mybir.AluOpType.add)

                    # heads: merged QK+bias for pair p, then PV/normalize p-1
                    pTs = {}
                    psos = {}

                    def emit_qk(h):
                        pss = psb.tile([128, 2 * N], F32, tag="ps")
                        for c in range(2):
                            nc.tensor.matmul(
                                pss[0:CH, c * N:(c + 1) * N],
                                lhsT=qk2[0:96, 12 + h,
                                         woff + c * CH:woff + c * CH + CH],
                                rhs=qk2[0:96, h, woff:woff + N],
                                start=True, stop=True)
                        pT = phd.tile([128, 2, 208], F8, tag="pT")
                        nc.scalar.activation(out=pT[0:CH, :, 0:N], in_=pss[0:CH, 0:2 * N],
                                             func=mybir.ActivationFunctionType.Exp)
                        pTs[h] = pT

                    def emit_pv(p):
                        pso = psb.tile([128, 2 * N], F32, tag="ps")
                        for h in (2 * p, 2 * p + 1):
                            b0 = (h % 2) * 64
                            pT = pTs.pop(h)
                            if b0 == 0:
                                # DoubleRow folds both key-chunks into one pass
                                nc.tensor.matmul(pso[0:64, 0:N],
                                                 lhsT=va[0:CH, 0:2, h * 64:(h + 1) * 64],
                                                 rhs=pT[0:CH, 0:2, 0:N],
                                                 perf_mode=DR, start=True, stop=True,
                                                 skip_group_check=True)
                                nc.tensor.matmul(pso[0:64, N:2 * N],
                                                 lhsT=va[0:CH, 0:2, DIM:DIM + 64],
                                                 rhs=pT[0:CH, 0:2, 0:N],
                                                 perf_mode=DR, start=True, stop=True,
                                                 skip_group_check=True)
                            else:
                                # walrus rejects DoubleRow + col-offset
                                # tile_position; plain fp8 per chunk instead
                                for c in range(2):
                                    nc.tensor.matmul(pso[64:128, 0:N],
                                                     lhsT=va[0:CH, c, h * 64:(h + 1) * 64],
                                                     rhs=pT[0:CH, c, 0:N],
                                                     start=(c == 0), stop=(c == 1),
                                                     skip_group_check=True)
                                    nc.tensor.matmul(pso[64:128, N:2 * N],
                                                     lhsT=va[0:CH, c, DIM:DIM + 64],
                                                     rhs=pT[0:CH, c, 0:N],
                                                     start=(c == 0), stop=(c == 1),
                                                     skip_group_check=True)
                        psos[p] = pso

                    def emit_norm(p):
                        pso = psos.pop(p)
                        rb = phd.tile([128, N], F32, tag="rb")
                        nc.vector.reciprocal(out=rb[:], in_=pso[:, N:2 * N])
                        nc.vector.tensor_mul(out=att[:, p, woff:woff + N],
                                             in0=pso[:, 0:N], in1=rb[:])

                    for p in range(6):
                        emit_qk(2 * p)
                        emit_qk(2 * p + 1)
                        if p >= 1:
                            emit_pv(p - 1)
                            emit_norm(p - 1)
                    emit_pv(5)
                    emit_norm(5)

                    # proj (fp8 DoubleRow) -> ow, then unpartition scatter
                    ow = pow_.tile([128, 2, DIM], BF16, tag="ow")
                    for c, coff in ((0, 0), (1, CH)):
                        pp0 = psb.tile([128, 384], F32, tag="ps")
                        pp1 = psb.tile([128, 384], F32, tag="ps")
                        for kp in range(3):
                            nc.tensor.matmul(
                                pp0[0:CH, :],
                                lhsT=att[:, 2 * kp:2 * kp + 2,
                                         woff + coff:woff + coff + CH],
                                rhs=wp_sb[:, 2 * kp:2 * kp + 2, 0:384],
                                perf_mode=DR,
                                start=(kp == 0), stop=(kp == 2))
                            nc.tensor.matmul(
                                pp1[0:CH, :],
                                lhsT=att[:, 2 * kp:2 * kp + 2,
                                         woff + coff:woff + coff + CH],
                                rhs=wp_sb[:, 2 * kp:2 * kp + 2, 384:768],
                                perf_mode=DR,
                                start=(kp == 0), stop=(kp == 2))
                        for half, psp in ((0, pp0), (1, pp1)):
                            if _CACHE.get('pb_zero'):
                                nc.scalar.activation(
                                    out=ow[0:CH, c, half * 384:(half + 1) * 384],
                                    in_=psp[0:CH, :],
                                    func=mybir.ActivationFunctionType.Copy,
                                    bias=0.0, scale=inv_w)
                            else:
                                nc.vector.scalar_tensor_tensor(
                                    out=ow[0:CH, c, half * 384:(half + 1) * 384],
                                    in0=psp[0:CH, :], scalar=inv_w,
                                    in1=pb_sb[0:CH, half * 384:(half + 1) * 384],
                                    op0=mybir.AluOpType.mult,
                                    op1=mybir.AluOpType.add)
                    wr, wc = _win_rc(w)
                    for c in range(2):
                        nc.gpsimd.dma_start(
                            out=at_img[wr * WS + c * 7:wr * WS + c * 7 + 7,
                                       wc * WS:wc * WS + WS, :],
                            in_=ow[0:CH, c, :])

            def emit_c_group(g):
                """Phase C for token tiles 4g..4g+3 (512 tokens = 8 image
                rows): residual, LN2, MLP, out.  4-tile grouping halves the
                gelu / bn / DMA fixed costs vs per-2-tile groups."""
                xg = pg.tile([128, 4, DIM], F32, tag="xg")
                ag = pag.tile([128, 4, DIM], BF16, tag="ag")
                nc.gpsimd.dma_start(out=xg[:], in_=x_pt[:, 4 * g:4 * g + 4, :])
                for a in range(4):
                    r0 = 8 * g + 2 * a
                    nc.gpsimd.dma_start(out=ag[:, a, :],
                                        in_=at_img[r0:r0 + 2, 0:HH, :])
                # x2 = x + attn (in place into xg)
                nc.vector.tensor_add(out=xg[:, :, :], in0=xg[:, :, :], in1=ag[:, :, :])
                xn2t = pgx.tile([128, 6, 512], F8, tag="xn2t")
                mvc = pg.tile([128, 2, 4], F32, tag="mvc")
                for s in range(4):
                    st = pg.tile([128, 2, 6], F32, tag="stC")
                    for sub in range(2):
                        nc.vector.bn_stats(out=st[:, sub, :],
                                           in_=xg[:, s, sub * 384:(sub + 1) * 384])
                    nc.vector.bn_aggr(out=mvc[:, :, s], in_=st[:])
                sdc = pg.tile([128, 4], F32, tag="sdC")
                nc.scalar.activation(out=sdc[:], in_=mvc[:, 1, :],
                                     func=mybir.ActivationFunctionType.Sqrt,
                                     bias=eps_t[:], scale=1.0)
                rsc = pg.tile([128, 4], F32, tag="rsC")
                nc.vector.reciprocal(out=rsc[:], in_=sdc[:])
                for s in range(4):
                    xn2b = pg.tile([128, DIM], F8, tag="xn2b")
                    nc.vector.tensor_scalar(out=xn2b[:, :], in0=xg[:, s, :],
                                            scalar1=mvc[:, 0, s:s + 1],
                                            scalar2=rsc[:, s:s + 1],
                                            op0=mybir.AluOpType.subtract,
                                            op1=mybir.AluOpType.mult)
                    if not _CACHE.get('b2_zero'):
                        nc.vector.tensor_add(out=xg[:, s, :], in0=xg[:, s, :],
                                             in1=b2_sb[:])
                    for j in range(6):
                        pt = ptb.tile([128, 128], F32, tag="pt")
                        nc.tensor.matmul(pt[:, :],
                                         lhsT=xn2b[:, j * 128:(j + 1) * 128],
                                         rhs=ident[:, :], start=True, stop=True)
                        nc.vector.tensor_copy(out=xn2t[:, j, s * 128:(s + 1) * 128],
                                              in_=pt[:, :])
                h1t = ph.tile([128, 24, 512], BF16, tag="h1t")
                for oc in range(24):
                    psh = psb.tile([128, 512], F32, tag="ps")
                    for kp in range(3):
                        nc.tensor.matmul(
                            psh[:, :],
                            lhsT=w1_sb[:, 2 * kp:2 * kp + 2, oc * 128:(oc + 1) * 128],
                            rhs=xn2t[:, 2 * kp:2 * kp + 2, :],
                            perf_mode=DR,
                            start=(kp == 0), stop=(kp == 2))
                    if sig_gelu:
                        # CoreSim lacks Gelu; x*sigmoid(1.702x) validates shapes
                        hpre = pg.tile([128, 512], BF16, tag="hpre")
                        nc.scalar.activation(out=hpre[:], in_=psh[:, :],
                                             func=mybir.ActivationFunctionType.Identity,
                                             bias=b1_sb[:, oc:oc + 1], scale=inv_w)
                        sg = pg.tile([128, 512], BF16, tag="sg")
                        nc.scalar.activation(out=sg[:], in_=hpre[:],
                                             func=mybir.ActivationFunctionType.Sigmoid,
                                             bias=0.0, scale=1.702)
                        nc.vector.tensor_mul(out=h1t[:, oc, :], in0=hpre[:], in1=sg[:])
                    else:
                        nc.scalar.activation(out=h1t[:, oc, :], in_=psh[:, :],
                                             func=mybir.ActivationFunctionType.Gelu,
                                             bias=b1_sb[:, oc:oc + 1], scale=inv_w)
                for sp in range(2):
                    yo = py.tile([128, 2, DIM], F32, tag="yo")
                    for ss in range(2):
                        s = 2 * sp + ss
                        pf0 = psb.tile([128, 384], F32, tag="ps")
                        pf1 = psb.tile([128, 384], F32, tag="ps")
                        for kt in range(24):
                            nc.tensor.matmul(
                                pf0[:, :],
                                lhsT=h1t[:, kt, s * 128:(s + 1) * 128],
                                rhs=w2_sb[:, kt, 0:384],
                                start=(kt == 0), stop=(kt == 23))
                            nc.tensor.matmul(
                                pf1[:, :],
                                lhsT=h1t[:, kt, s * 128:(s + 1) * 128],
                                rhs=w2_sb[:, kt, 384:768],
                                start=(kt == 0), stop=(kt == 23))
                        for half, psf in ((0, pf0), (1, pf1)):
                            nc.vector.tensor_add(
                                out=yo[:, ss, half * 384:(half + 1) * 384],
                                in0=psf[:, :],
                                in1=xg[:, s, half * 384:(half + 1) * 384])
                    nc.gpsimd.dma_start(out=y_pt[:, 4 * g + 2 * sp:4 * g + 2 * sp + 2, :],
                                        in_=yo[:])

            # phase C double-group G covers image rows 8G..8G+8; ready once
            # the band containing its last row is done.  Groups are POPPED
            # one band later (after that band's last window group) so band
            # boundaries always have PE-ready work.
            c_ready = {0: [0], 1: [1, 2], 2: [3, 4], 3: [5, 6], 4: [7]}
            pending = []

            emit_ln1_band(0)
            emit_attn_weights()
            for band in range(5):
                w0 = band * NWS
                emit_window_group((w0, w0 + 1))
                if band == 0:
                    emit_zero_pad()
                    emit_bulk_weights()
                # overlap next band's LN1 (DVE/DMA) with this band's windows
                if band < 4:
                    emit_ln1_band(band + 1)
                emit_window_group((w0 + 2, w0 + 3))
                emit_window_group((w0 + 4,))
                while pending:
                    emit_c_group(pending.pop(0))
                pending.extend(c_ready[band])
            for g in pending:
                emit_c_group(g)

    if os.environ.get('KERNEL_NOLDDEDUP') != '1':
        _dedup_ldweights(nc)
    if os.environ.get('KERNEL_SIM') != '1':
        _split_waits(nc)
    _CACHE['nc'] = nc
    return nc


def _host_prep(inputs):
    """Fold LN affines into matmul weights, build rel-pos operands."""
    f32 = np.float32
    x = np.asarray(inputs['x'], f32)
    q_idx = np.asarray(inputs['q_idx']).astype(np.int64)
    k_idx = np.asarray(inputs['k_idx']).astype(np.int64)
    ln1_w = np.asarray(inputs['ln1_w'], f32); ln1_b = np.asarray(inputs['ln1_b'], f32)
    ln2_w = np.asarray(inputs['ln2_w'], f32); ln2_b = np.asarray(inputs['ln2_b'], f32)
    qkv_w = np.asarray(inputs['qkv_w'], f32); qkv_b = np.asarray(inputs['qkv_b'], f32)
    proj_w = np.asarray(inputs['proj_w'], f32); proj_b = np.asarray(inputs['proj_b'], f32)
    mlp_w1 = np.asarray(inputs['mlp_w1'], f32); mlp_b1 = np.asarray(inputs['mlp_b1'], f32)
    mlp_w2 = np.asarray(inputs['mlp_w2'], f32); mlp_b2 = np.asarray(inputs['mlp_b2'], f32)
    rel_h = np.asarray(inputs['rel_h'], f32); rel_w = np.asarray(inputs['rel_w'], f32)

    scale = HD ** -0.5
    Wqkv = ln1_w[:, None] * qkv_w
    bqkv = ln1_b @ qkv_w + qkv_b
    Wqkv = Wqkv.copy(); bqkv = bqkv.copy()
    Wqkv[:, :DIM] *= scale
    bqkv[:DIM] *= scale
    W1 = ln2_w[:, None] * mlp_w1
    b1 = ln2_b @ mlp_w1 + mlp_b1

    coords = np.arange(WS)[:, None] - np.arange(WS)[None, :] + (WS - 1)
    Sh = rel_h[coords].sum(-1).astype(f32)
    Sw = rel_w[coords].sum(-1).astype(f32)

    qr, qc = q_idx // WS, q_idx % WS
    kr, kc = k_idx // WS, k_idx % WS
    nb = q_idx.shape[0]
    Eq = np.zeros((nb, EB, N), f32)
    Eq[:, 0:WS, :] = np.take(Sh, qr, axis=0).transpose(0, 2, 1)
    Eq[:, WS:2 * WS, :] = np.take(Sw, qc, axis=0).transpose(0, 2, 1)
    Ek = np.zeros((nb, EB, N), f32)
    bi = np.arange(nb)[:, None]
    ar = np.arange(N)[None, :]
    Ek[bi, kr, ar] = 1.0
    Ek[bi, WS + kc, ar] = 1.0

    bf = ml_dtypes.bfloat16
    f8 = ml_dtypes.float8_e4m3fn
    shared = {
        "wqk": np.ascontiguousarray(Wqkv[:, :2 * DIM] * WSCALE).astype(f8),
        "wv": np.ascontiguousarray(Wqkv[:, 2 * DIM:] * WSCALE).astype(f8),
        "bqk": np.ascontiguousarray(bqkv[:2 * DIM].reshape(12, 128).T),
        "vb": np.ascontiguousarray(bqkv[2 * DIM:].reshape(1, DIM)),
        "wp": np.ascontiguousarray(proj_w * WSCALE).astype(f8),
        "pb": proj_b.reshape(1, DIM).copy(),
        "w1": np.ascontiguousarray(W1 * WSCALE).astype(f8),
        "b1": np.ascontiguousarray(b1.reshape(24, 128).T),
        "w2": mlp_w2.astype(bf),
        "b2": mlp_b2.reshape(1, DIM).copy(),
    }
    Eq = Eq.astype(f8).reshape(B, NW, NH, EB, N)
    Ek = Ek.astype(f8).reshape(B, NW, NH, EB, N)
    # regroup per window-group (pair, pair, single per band), windows'
    # key columns contiguous on the last axis
    EqG = np.zeros((B, 15, NH, EB, 2 * N), f8)
    EkG = np.zeros((B, 15, NH, EB, 2 * N), f8)
    for band in range(5):
        for gi, ws_ in enumerate(((0, 1), (2, 3), (4,))):
            g = band * 3 + gi
            for wi, wo in enumerate(ws_):
                w = band * NWS + wo
                EqG[:, g, :, :, wi * N:(wi + 1) * N] = Eq[:, w]
                EkG[:, g, :, :, wi * N:(wi + 1) * N] = Ek[:, w]
    in_maps = []
    for b in range(B):
        m = dict(shared)
        m["x"] = np.ascontiguousarray(x[b].reshape(NTOK, DIM))
        m["xbf"] = np.ascontiguousarray(x[b].reshape(NTOK, DIM)).astype(bf)
        m["eq"] = np.ascontiguousarray(EqG[b])
        m["ek"] = np.ascontiguousarray(EkG[b])
        in_maps.append(m)
    return in_maps


def kernel(**inputs):
    in_maps = _host_prep(inputs)
    if 'nc' not in _CACHE:
        _CACHE['pb_zero'] = not np.any(np.asarray(in_maps[0]['pb'], np.float32))
        _CACHE['b2_zero'] = not np.any(np.asarray(in_maps[0]['b2'], np.float32))
    nc = _build()
    trace = os.environ.get('KERNEL_TRACE') == '1'
    if trace:
        _install_ntff_hook()
    res = run_bass_kernel_spmd(nc, in_maps, list(range(B)), trace=trace)
    if trace and res.exec_time_ns is not None:
        print(f"HW exec time: {res.exec_time_ns} ns")
        _CACHE['exec_time_ns'] = res.exec_time_ns
    _CACHE['last_results'] = res
    out = np.stack([np.asarray(res.results[b]["y"]).reshape(HH, WW, DIM)
                    for b in range(B)])
    return out.astype(np.float32)
